# revision 1
# baseline (speedup 1.0000x reference)
"""MobileMQA Trainium2 kernel v2 (8 NeuronCores, SPMD).

Reference computation (per batch b of 2):
  q  = x @ wq + bq                         [1024 tok, 512]
  kv = x @ wkv + bkv                       [1024 tok, 1024]
  kv = depthwise3x3_s2_same(kv) + dw_bias  [256 sp, 1024]
  k, v = split(kv)  -> shared-KV length M=2048 (channel fold)
  attn = softmax(q @ k^T * 0.125); out = attn @ v
  y = out @ wo + bo

Sharding: core c handles batch b=c//4, query chunk j=c%4 (256 tokens).
KV path (proj+conv) replicated across the 4 cores of a batch (MQA).

v2 design vs baseline (88150 -> 67765 ns in TimelineSim):
  - fp16 operands on the PE (1 cycle/row at any N; 11-bit mantissa);
    exp/V/attn in bf16 where range requires it. rel_err ~6.5e-3.
  - Dual-engine softmax exp: most score tiles on ACT (table exp),
    every 4th plus the tail stretch (units 54/58/62, full ACT/DVE
    alternation at the end) on DVE via the Schraudolph bit trick
    (int16(s*23.083 + 16248.8) bitcast to bf16), so the serial ACT
    chain gates neither the attention cadence nor the finish.
  - Fully fused proj+conv / attention pipeline: attention units (mt,
    quarter) for ch-tile t are emitted interleaved with ch-tile t+1's
    proj/conv micro-ops; K-proj rotates through the 4 "st" PSUM slots in
    parallel with the V chain on its own slot.
  - AV matmul flipped: exp(S) chunks are the stationary operand, V_aug
    [128m, 65] the moving one -> 65-row streams instead of 512.
  - Scores single-half (kT2/qT2 [64, 2048], no partition duplication).
  - Conv: taps 0-6 as PE diag-matmuls (host-built fp16 diagonals via
    DMA); taps 7-8 + bias plane accumulate in an SBUF side-buffer on DVE
    and merge in the K/V copy-stt (gpsimd cannot access PSUM, so all
    cross-partition copies stage through SBUF).
  - PSUM zero-regions are 2KB: the 16 interleaved AV accumulators share
    one pre-zeroed group (dummy matmuls, then start=False streaming).
  - PE warm-up matmuls during the input-DMA window beat the p-state ramp.
  - Normalization via per-partition 1/z scalar (z = ones-column of AV),
    then per-ch-block transpose + y-projection accumulation; bo is folded
    into yproj as a 1-partition matmul so the tail is two parallel plain
    copies (DVE || ACT) plus the output DMAs.
"""
import sys

for _p in ("/opt/trn_rl_repo", "/opt/trn_rl_repo/concourse"):
    if _p not in sys.path:
        sys.path.insert(0, _p)

import numpy as np

import concourse.bass as bass
import concourse.mybir as mybir
import concourse.tile as tile
from concourse import bacc
from concourse.bass_utils import run_bass_kernel_spmd
from concourse.masks import make_identity

F32 = mybir.dt.float32
F16 = mybir.dt.float16
BF16 = mybir.dt.bfloat16
AF = mybir.ActivationFunctionType
ALU = mybir.AluOpType

DIM = 512
NH = 8
HD = 64
B, H, W = 2, 32, 32
L = H * W            # 1024 tokens per batch
KH = KW = 16
NS = KH * KW         # 256 conv-output spatial positions
M = NS * NH          # 2048 shared-KV positions
CH = 2 * DIM         # 1024 kv channels
SCALE = HD ** -0.5   # 0.125
PADW = 33            # padded conv input row (32 + 1 SAME pad after)
NPAD = PADW * PADW   # 1089

_NC_CACHE = {}


def _build_program():
    nc = bacc.Bacc(None)

    xT_d = nc.dram_tensor("xT", [DIM, L], F16, kind="ExternalInput")
    xTc_d = nc.dram_tensor("xTc", [DIM, 256], F16, kind="ExternalInput")
    wkv_d = nc.dram_tensor("wkv", [DIM, CH], F16, kind="ExternalInput")
    wq_d = nc.dram_tensor("wq", [DIM, DIM], F16, kind="ExternalInput")
    wo_d = nc.dram_tensor("wo", [DIM, DIM], F16, kind="ExternalInput")
    bpl_d = nc.dram_tensor("bpl", [CH, NS], F16, kind="ExternalInput")
    dgw_d = nc.dram_tensor("dgw", [128, 56 * 128], F16, kind="ExternalInput")
    # cst cols: 0-3 bq tiles, 4-7 bo tiles
    cst_d = nc.dram_tensor("cst", [128, 40], F32, kind="ExternalInput")
    bo16_d = nc.dram_tensor("bo16", [1, DIM], F16, kind="ExternalInput")
    y_d = nc.dram_tensor("y", [DIM, 256], F32, kind="ExternalOutput")
    import os as _os
    _dbg = _os.environ.get("BASSDBG") == "1"
    if _dbg:
        kT2_o = nc.dram_tensor("kT2o", [64, M], F16, kind="ExternalOutput")
        qT2_o = nc.dram_tensor("qT2o", [64, M], F16, kind="ExternalOutput")
        vaug_o = nc.dram_tensor("vaugo", [128, 16 * (HD + 1)], BF16,
                                kind="ExternalOutput")
        asb_o = nc.dram_tensor("asbo", [128, 16 * HD], F16,
                               kind="ExternalOutput")

    with tile.TileContext(nc) as tc:
        with tc.tile_pool(name="wp", bufs=1) as wp, \
             tc.tile_pool(name="expp", bufs=8) as expp, \
             tc.tile_pool(name="tmpq", bufs=2) as tmppool, \
             tc.tile_pool(name="ps", bufs=1, space="PSUM") as ps:

            # ---------------- input DMAs (priority order) ----------------
            cst = wp.tile([128, 40], F32, tag="cst")

            xTc = wp.tile([128, 4, 256], F16, tag="xTc")
            xTc_r = xTc_d[:, :].rearrange("(k p) t -> p k t", p=128)
            wq = wp.tile([128, 4, DIM], F16, tag="wq")
            wq_r = wq_d[:, :].rearrange("(k p) c -> p k c", p=128)
            xT = wp.tile([128, 4, L], F16, tag="xT")
            wkv = wp.tile([128, 4, CH], F16, tag="wkv")
            dgw = wp.tile([128, 56, 128], F16, tag="dgw")
            bpl = wp.tile([128, 8, NS], F16, tag="bpl")
            xT_r = xT_d[:, :].rearrange("(k p) t -> p k t", p=128)
            wkv_r = wkv_d[:, :].rearrange("(k p) c -> p k c", p=128)
            bpl_r = bpl_d[:, :].rearrange("(t p) s -> p t s", p=128)

            # critical-path order: K0/V0 proj operands, then q operands
            nc.sync.dma_start(out=wkv[:, :, 0:256], in_=wkv_r[:, :, 0:256])
            nc.sync.dma_start(out=xT[:, 0:2, 0:512], in_=xT_r[:, 0:2, 0:512])
            nc.sync.dma_start(out=xT[:, 2:4, 0:512], in_=xT_r[:, 2:4, 0:512])
            nc.sync.dma_start(out=xTc, in_=xTc_r)
            nc.sync.dma_start(out=wq, in_=wq_r)
            nc.sync.dma_start(out=cst, in_=cst_d[:, :])
            nc.sync.dma_start(out=xT[:, :, 512:L], in_=xT_r[:, :, 512:L])
            nc.sync.dma_start(out=bpl[:, 0:2, :], in_=bpl_r[:, 0:2, :])
            nc.sync.dma_start(out=dgw[:, 0:14, :],
                              in_=dgw_d[:, 0:14 * 128])
            for t in range(1, 4):
                d0 = 2 * t
                nc.sync.dma_start(
                    out=wkv[:, :, d0 * 128:(d0 + 2) * 128],
                    in_=wkv_r[:, :, d0 * 128:(d0 + 2) * 128])
                nc.sync.dma_start(out=bpl[:, d0:d0 + 2, :],
                                  in_=bpl_r[:, d0:d0 + 2, :])
                nc.sync.dma_start(out=dgw[:, 7 * d0:7 * (d0 + 2), :],
                                  in_=dgw_d[:, 7 * d0 * 128:7 * (d0 + 2) * 128])
            bo16 = wp.tile([1, DIM], F16, tag="bo16")
            nc.sync.dma_start(out=bo16, in_=bo16_d[:, :])
            ones16 = wp.tile([1, 256], F16, tag="ones16")
            nc.vector.memset(ones16, 1.0)
            wo = wp.tile([128, 4, DIM], F16, tag="wo")
            nc.sync.dma_start(out=wo,
                              in_=wo_d[:, :].rearrange("(k p) c -> p k c", p=128))

            # ---------------- persistent SBUF state ----------------
            ident = wp.tile([128, 128], F16, tag="ident")
            make_identity(nc, ident)
            # preload exp ACT table during the DMA window
            warm = wp.tile([1, 1], F32, tag="warm")
            nc.vector.memset(warm, 0.0)
            nc.scalar.activation(warm[:, :], warm[:, :], AF.Exp)

            kT2 = wp.tile([64, M], F16, tag="kT2")
            qT2 = wp.tile([64, M], F16, tag="qT2")
            vaug = wp.tile([128, 16, HD + 1], BF16, tag="vaug")
            nc.vector.memset(vaug[:, :, HD:HD + 1], 1.0)
            attnT = wp.tile([128, 4, 256], F16, tag="attnT")
            attn_sb = wp.tile([128, 16, HD], F16, tag="attn_sb")
            zr = wp.tile([128, 16], F32, tag="zr")
            vacc = wp.tile([128, NS], F16, tag="vacc")
            ysb = wp.tile([128, 4, 256], F32, tag="ysb")

            # two explicit conv-input buffers; pad cells zeroed once
            kvsb = []
            for i in range(2):
                kb = wp.tile([128, NPAD], F16, tag=f"kvsb{i}",
                             name=f"kvsb{i}")
                nc.vector.memset(kb[:, :], 0.0)
                kvsb.append(kb)

            # PE warm-up: keep the array busy through the DMA window so the
            # p-state ramp completes before the real matmuls arrive.
            for _w in range(4):
                wmm = ps.tile([128, 512], F32, tag="kvp", bufs=1)
                nc.tensor.matmul(wmm[:, :], ident[:, :], kvsb[0][:, 0:512],
                                 start=True, stop=True)

            # ---------------- helpers ----------------
            def kv_proj_half(dg, nh):
                """kv proj for ch-tile dg, token half nh -> PSUM [128, 512]."""
                kvp = ps.tile([128, 512], F32, tag="kvp", bufs=1)
                for k in range(4):
                    nc.tensor.matmul(kvp[:, :],
                                     wkv[:, k, dg * 128:(dg + 1) * 128],
                                     xT[:, k, nh * 512:(nh + 1) * 512],
                                     start=(k == 0), stop=(k == 3))
                return kvp

            def pad_copy(dg, nh, kvp, eng):
                kb = kvsb[dg % 2]
                dst = bass.AP(tensor=kb.tensor,
                              offset=kb.offset + PADW * 16 * nh,
                              ap=[kb.ap[0], [PADW, 16], [1, 32]])
                src = kvp[:, :].rearrange("p (a b) -> p a b", b=32)
                if eng is nc.scalar:
                    eng.copy(dst, src)
                else:
                    eng.tensor_copy(dst, src)

            tmpp = tmppool

            def conv(dg):
                """Conv taps 0-6 on PE (PSUM cvp); taps 7,8 + bias into an
                SBUF side-accumulator (DVE then gpsimd), merged at the
                K/V stt."""
                kb = kvsb[dg % 2]
                tmp = tmpp.tile([128, NS], F32, tag="tmp")
                for tap, eng in ((7, nc.vector), (8, nc.vector)):
                    dy, dx = tap // 3, tap % 3
                    win = bass.AP(tensor=kb.tensor,
                                  offset=kb.offset + PADW * dy + dx,
                                  ap=[kb.ap[0], [2 * PADW, KH], [2, KW]])
                    col = 8 + (tap - 5) * 8 + dg
                    in1 = bpl[:, dg, :] if tap == 7 else tmp[:, :]
                    eng.scalar_tensor_tensor(
                        tmp[:, :], win, cst[:, col:col + 1], in1,
                        op0=ALU.mult, op1=ALU.add)
                cvp = ps.tile([128, NS], F32, tag="kvp", bufs=1)
                for tap in range(7):
                    dy, dx = tap // 3, tap % 3
                    win = bass.AP(tensor=kb.tensor,
                                  offset=kb.offset + PADW * dy + dx,
                                  ap=[kb.ap[0], [2 * PADW, KH], [2, KW]])
                    nc.tensor.matmul(cvp[:, :], dgw[:, dg * 7 + tap, :], win,
                                     start=(tap == 0), stop=(tap == 6))
                return cvp, tmp

            def k_tile(t, cvp, tmp):
                # gpsimd cannot read PSUM: merge to SBUF, then cross-copy
                kacc = tmpp.tile([128, NS], F16, tag="kacc")
                nc.vector.scalar_tensor_tensor(
                    kacc[:, :], cvp[:, :], 1.0, tmp[:, :],
                    op0=ALU.mult, op1=ALU.add)
                nc.gpsimd.tensor_copy(kT2[:, (2 * t) * 256:(2 * t + 1) * 256],
                                      kacc[0:64, :])
                nc.gpsimd.tensor_copy(
                    kT2[:, (2 * t + 1) * 256:(2 * t + 2) * 256],
                    kacc[64:128, :])

            def v_tile(t, cvp, tmp):
                nc.vector.scalar_tensor_tensor(
                    vacc[:, :], cvp[:, :], 1.0, tmp[:, :],
                    op0=ALU.mult, op1=ALU.add)
                for gi in range(2):
                    vt = ps.tile([128, 128], F16, tag="kvp", bufs=1)
                    for sh in range(2):
                        nc.tensor.transpose(
                            vt[:, sh * 64:(sh + 1) * 64],
                            vacc[gi * 64:(gi + 1) * 64,
                                 sh * 128:(sh + 1) * 128],
                            ident[gi * 64:(gi + 1) * 64,
                                  gi * 64:(gi + 1) * 64])
                    mt0 = 4 * t + 2 * gi
                    nc.vector.tensor_copy(
                        vaug[:, mt0:mt0 + 2, 0:HD],
                        vt[:, :].rearrange("p (a b) -> p a b", b=64))

            av = ps.tile([128, 16, HD + 1], F32, tag="av", bufs=1)
            # pre-zero the whole av region (PSUM zero-regions are 2KB; the
            # 16 interleaved 260B accumulation groups must share one group)
            av_flat = bass.AP(tensor=av.tensor, offset=av.offset,
                              ap=[av.ap[0], [1, 16 * (HD + 1)]])
            for z0 in (0, 512):
                zmm = bass.AP(tensor=av.tensor, offset=av.offset + z0,
                              ap=[av.ap[0], [1, 512]])
                nc.tensor.matmul(zmm, ident[:, :], kvsb[0][:, 0:512],
                                 start=True, stop=True, skip_group_check=True)
            zmm2 = bass.AP(tensor=av.tensor, offset=av.offset + 1024,
                           ap=[av.ap[0], [1, 16]])
            nc.tensor.matmul(zmm2, ident[:, :], kvsb[0][:, 0:16],
                             start=True, stop=True, skip_group_check=True)

            ucount = [0]

            def sc_unit(mt, uq):
                st = ps.tile([128, 512], F32, tag="st", bufs=4)
                nc.tensor.matmul(st[:, :],
                                 kT2[:, mt * 128:(mt + 1) * 128],
                                 qT2[:, uq * 512:(uq + 1) * 512],
                                 start=True, stop=True)
                ucount[0] += 1
                if ucount[0] % 4 == 0 or ucount[0] in (54, 58, 62):
                    # Schraudolph exp on DVE: bf16 bits = s*scale*184.66 +
                    # (16256 - 7.22); int16 convert, bitcast view as bf16.
                    exd = expp.tile([128, 512], mybir.dt.int16, tag="ex",
                                    name="exd")
                    nc.vector.tensor_scalar(
                        exd[:, :], st[:, :], 23.08294, 16248.78,
                        op0=ALU.mult, op1=ALU.add)
                    return exd.bitcast(BF16)
                ex = expp.tile([128, 512], BF16, tag="ex")
                nc.scalar.activation(ex[:, :], st[:, :], AF.Exp,
                                     scale=float(SCALE))
                return ex

            def av_unit(mt, uq, ex):
                for j in range(4):
                    nc.tensor.matmul(av[:, uq * 4 + j, :],
                                     ex[:, j * 128:(j + 1) * 128],
                                     vaug[:, mt, :],
                                     start=False, stop=(mt == 15),
                                     skip_group_check=True)

            # ---------------- q projection (emitted via emit_qproj) --------
            def emit_qproj(t):
                qp = ps.tile([128, 512], F32, tag="st", bufs=4)
                qpv = qp[:, 0:256]
                for k in range(4):
                    nc.tensor.matmul(qpv, wq[:, k, t * 128:(t + 1) * 128],
                                     xTc[:, k, :],
                                     start=(k == 0), stop=(k == 3))
                nc.vector.tensor_scalar_add(qpv, qpv, cst[:, t:t + 1])
                nc.vector.tensor_copy(qT2[:, (2 * t) * 256:(2 * t + 1) * 256],
                                      qp[0:64, 0:256])
                qstage = tmpp.tile([128, 256], F16, tag="kacc", name="qstage")
                nc.vector.tensor_copy(qstage[64:128, :], qp[64:128, 0:256])
                nc.gpsimd.tensor_copy(qT2[:, (2 * t + 1) * 256:(2 * t + 2) * 256],
                                      qstage[64:128, :])

            # ---------------- fused P1/P2 pipeline ----------------
            # K-proj halves rotate through the "st" slots (parallel to the
            # V chain on "kvp"); t=0 pads on ACT (idle pre-exp), later K
            # pads on DVE, V pads on gpsimd.
            kvp_store = {}

            def pad_eng(dg):
                # gpsimd cannot access PSUM; split PSUM reads DVE/ACT
                if dg < 2:
                    return nc.scalar
                return nc.vector

            def proj_op(dg, nh):
                if dg % 2 == 0:  # K tile -> st slots
                    kvp = ps.tile([128, 512], F32, tag="st", bufs=4,
                                  name="kvpk")
                else:
                    kvp = ps.tile([128, 512], F32, tag="kvp", bufs=1)
                for k in range(4):
                    nc.tensor.matmul(kvp[:, :],
                                     wkv[:, k, dg * 128:(dg + 1) * 128],
                                     xT[:, k, nh * 512:(nh + 1) * 512],
                                     start=(k == 0), stop=(k == 3))
                kvp_store[(dg, nh)] = kvp

            def pad_op(dg, nh):
                pad_copy(dg, nh, kvp_store.pop((dg, nh)), pad_eng(dg))

            def p1_ops(t):
                dgk, dgv = 2 * t, 2 * t + 1
                mk = lambda f, *a: (lambda: f(*a))
                cstore = {}

                def conv_op(dg):
                    cstore[dg] = conv(dg)

                def ktail(tt):
                    k_tile(tt, *cstore.pop(2 * tt))

                def vtail(tt):
                    v_tile(tt, *cstore.pop(2 * tt + 1))

                return [
                    mk(proj_op, dgk, 0), mk(pad_op, dgk, 0),
                    mk(proj_op, dgv, 0), mk(pad_op, dgv, 0),
                    mk(proj_op, dgk, 1), mk(pad_op, dgk, 1),
                    mk(proj_op, dgv, 1), mk(pad_op, dgv, 1),
                    mk(conv_op, dgk), mk(ktail, t),
                    mk(conv_op, dgv), mk(vtail, t),
                ]

            ops0 = p1_ops(0)
            for op in ops0[0:4]:     # K0/V0 first halves
                op()
            emit_qproj(0)
            for op in ops0[4:8]:     # second halves
                op()
            emit_qproj(1)
            emit_qproj(2)
            for op in ops0[8:10]:    # conv K0
                op()
            emit_qproj(3)
            for op in ops0[10:12]:   # conv V0
                op()
            # Unit stream with av lagging sc by one; p1 micro-ops of tile
            # t+1 round-robined between tile t's units.
            pending = []
            for t in range(4):
                us = [(mt, uq) for mt in range(4 * t, 4 * t + 4)
                      for uq in range(4)]
                chunks = p1_ops(t + 1) if t < 3 else []
                nu, nch = len(us), len(chunks)
                ci = 0
                for i, u in enumerate(us):
                    mt, uq = u
                    ex = sc_unit(mt, uq)
                    pending.append((mt, uq, ex))
                    if len(pending) > 2:
                        av_unit(*pending.pop(0))
                    want = ((i + 1) * nch) // nu
                    while ci < want:
                        chunks[ci]()
                        ci += 1
            for p_ in pending:
                av_unit(*p_)

            # ---------------- normalize + attnT + y proj ----------------
            # chunks 0-7 (half 0) complete one unit before 8-15; normalize,
            # transpose and accumulate y per ch-block as results land.
            def slot_of(chunk):
                h, lh = chunk // 2, chunk % 2
                return 4 * (h // 2) + 2 * lh + (h % 2)

            def norm_chunk(chunk):
                slot = slot_of(chunk)
                nc.vector.reciprocal(zr[:, chunk:chunk + 1],
                                     av[:, chunk, HD:HD + 1])
                if chunk % 2 == 0:
                    nc.vector.tensor_scalar_mul(attn_sb[:, slot, :],
                                                av[:, chunk, 0:HD],
                                                zr[:, chunk:chunk + 1])
                else:
                    nc.scalar.mul(attn_sb[:, slot, :], av[:, chunk, 0:HD],
                                  zr[:, chunk:chunk + 1])

            yps = [None, None]

            def attnT_block(kk):
                for lh in range(2):
                    vt2 = ps.tile([128, 128], F16, tag="kvp", bufs=1)
                    s0 = 4 * kk + 2 * lh
                    src_ap = bass.AP(
                        tensor=attn_sb.tensor,
                        offset=attn_sb.offset + s0 * HD,
                        ap=[attn_sb.ap[0], [1, 128]])
                    nc.tensor.transpose(vt2[:, :], src_ap, ident[:, :])
                    if kk < 2:
                        nc.scalar.copy(
                            attnT[:, kk, lh * 128:(lh + 1) * 128], vt2[:, :])
                    else:
                        nc.vector.tensor_copy(
                            attnT[:, kk, lh * 128:(lh + 1) * 128], vt2[:, :])
                for m in range(4):
                    if yps[m // 2] is None:
                        yps[m // 2] = ps.tile([128, 512], F32, tag="st",
                                              bufs=4, name=f"yp{m // 2}")
                    nc.tensor.matmul(
                        yps[m // 2][:, (m % 2) * 256:(m % 2) * 256 + 256],
                        wo[:, kk, m * 128:(m + 1) * 128],
                        attnT[:, kk, :],
                        start=(kk == 0 and m % 2 == 0), stop=(kk == 3),
                        skip_group_check=True)
                    if kk == 0:
                        nc.tensor.matmul(
                            yps[m // 2][:, (m % 2) * 256:(m % 2) * 256 + 256],
                            bo16[:, m * 128:(m + 1) * 128],
                            ones16[:, :],
                            start=False, stop=False, skip_group_check=True)

            for chunk in range(8):
                norm_chunk(chunk)
            attnT_block(0)
            attnT_block(1)
            for chunk in range(8, 16):
                norm_chunk(chunk)
            attnT_block(2)
            attnT_block(3)

            if _dbg:
                nc.sync.dma_start(out=kT2_o[:, :], in_=kT2[:, :])
                nc.sync.dma_start(out=qT2_o[:, :], in_=qT2[:, :])
                nc.sync.dma_start(out=vaug_o[:, :],
                                  in_=vaug[:, :, :].rearrange("p a b -> p (a b)"))
                nc.sync.dma_start(out=asb_o[:, :],
                                  in_=attn_sb[:, :, :].rearrange("p a b -> p (a b)"))
            nc.vector.tensor_copy(ysb[:, 0:2, :],
                                  yps[0][:, :].rearrange("p (m t) -> p m t",
                                                         t=256))
            nc.scalar.copy(ysb[:, 2:4, :],
                           yps[1][:, :].rearrange("p (m t) -> p m t", t=256))
            for i in range(2):
                nc.sync.dma_start(
                    out=y_d[i * 256:(i + 1) * 256, :].rearrange(
                        "(m p) t -> p m t", p=128),
                    in_=ysb[:, 2 * i:2 * i + 2, :])

    nc.finalize()
    return nc


def _get_program():
    if "nc" not in _NC_CACHE:
        _NC_CACHE["nc"] = _build_program()
    return _NC_CACHE["nc"]


def _host_prep(x, wq, bq, wkv, bkv, dw_kernel, dw_bias, wo, bo):
    """Build the 8 per-core input maps."""
    x = np.ascontiguousarray(np.asarray(x, np.float32))
    wq16 = np.asarray(wq, np.float32).astype(np.float16)
    wo16 = np.asarray(wo, np.float32).astype(np.float16)
    bq = np.asarray(bq, np.float32)
    bkv = np.asarray(bkv, np.float32)
    dw_bias = np.asarray(dw_bias, np.float32)
    bo = np.asarray(bo, np.float32)
    dww = np.asarray(dw_kernel, np.float32).reshape(9, CH).T.copy()  # [1024, 9]

    # channel-tile processing order K0 V0 K1 V1 ... ; dg -> channel base
    dg_base = []
    for t in range(4):
        dg_base += [t * 128, DIM + t * 128]

    # wkv columns reordered to dg order
    wkv_f = np.asarray(wkv, np.float32)
    wkv16 = np.empty((DIM, CH), np.float16)
    for dg in range(8):
        b0 = dg_base[dg]
        wkv16[:, dg * 128:(dg + 1) * 128] = wkv_f[:, b0:b0 + 128]

    # bias plane: dw_bias + bkv * sum(valid taps), SAME padding aware,
    # rows in dg order
    oy = np.arange(KH)
    valid_y = (2 * oy[:, None] + np.arange(3)[None, :]) < H      # [16, 3]
    valid_x = valid_y.copy()
    wsum = np.zeros((CH, KH, KW), np.float32)
    for tap in range(9):
        dy, dx = tap // 3, tap % 3
        m2 = np.outer(valid_y[:, dy], valid_x[:, dx]).astype(np.float32)
        wsum += dww[:, tap][:, None, None] * m2[None, :, :]
    bpl_full = (dw_bias[:, None] + bkv[:, None] * wsum.reshape(CH, NS))
    bpl16 = np.empty((CH, NS), np.float16)
    for dg in range(8):
        b0 = dg_base[dg]
        bpl16[dg * 128:(dg + 1) * 128] = bpl_full[b0:b0 + 128]

    # conv diagonal weights [128, 72*128] fp16, dg-major then tap
    dgw = np.zeros((128, 56, 128), np.float16)
    idx = np.arange(128)
    for dg in range(8):
        b0 = dg_base[dg]
        for tap in range(7):
            dgw[idx, dg * 7 + tap, idx] = dww[b0 + idx, tap].astype(np.float16)
    dgw = dgw.reshape(128, 56 * 128)

    cst = np.zeros((128, 40), np.float32)
    cst[:, 0:4] = bq.reshape(4, 128).T
    cst[:, 4:8] = bo.reshape(4, 128).T
    for dg in range(8):
        b0 = dg_base[dg]
        for tap in range(5, 9):
            cst[:, 8 + (tap - 5) * 8 + dg] = dww[b0:b0 + 128, tap]

    in_maps = []
    for c in range(8):
        b, j = c // 4, c % 4
        xT = x[b].reshape(L, DIM).T.astype(np.float16)
        in_maps.append({
            "xT": np.ascontiguousarray(xT),
            "xTc": np.ascontiguousarray(xT[:, j * 256:(j + 1) * 256]),
            "wkv": wkv16, "wq": wq16, "wo": wo16,
            "bpl": bpl16, "dgw": dgw, "cst": cst,
            "bo16": bo.reshape(1, DIM).astype(np.float16),
        })
    return in_maps


def kernel(**inputs) -> np.ndarray:
    nc = _get_program()
    in_maps = _host_prep(**inputs)
    res = run_bass_kernel_spmd(nc, in_maps, core_ids=list(range(8)))
    out = np.zeros((B, H, W, DIM), np.float32)
    flat = out.reshape(B, L, DIM)
    for c in range(8):
        b, j = c // 4, c % 4
        flat[b, j * 256:(j + 1) * 256, :] = res.results[c]["y"].T
    return out



# revision 14
# speedup vs baseline: 1.0345x; 1.0345x over previous
"""MobileMQA Trainium2 kernel v3 (8 NeuronCores, SPMD).

Reference computation (per batch b of 2):
  q  = x @ wq + bq                         [1024 tok, 512]
  kv = x @ wkv + bkv                       [1024 tok, 1024]
  kv = depthwise3x3_s2_same(kv) + dw_bias  [256 sp, 1024]
  k, v = split(kv)  -> shared-KV length M=2048 (channel fold)
  attn = softmax(q @ k^T * 0.125); out = attn @ v
  y = out @ wo + bo

Sharding: core c handles batch b=c//4, query chunk j=c%4 (256 tokens).
KV path (proj+conv) replicated across the 4 cores of a batch (MQA).

v3 design vs v2 (67765 ns):
  - Tail restructured: last mt-group runs uq-major; per-uq normalize +
    attnT transposes pipeline INTO the attention stream (prev-uq blocks
    emitted between next-uq score units). y-proj is a tight kk-major
    burst at the end; bo folds into the final PSUM->SBUF copies via cst
    (bias matmuls + ones16/bo16 inputs dropped); per-half output DMAs
    issue as soon as their ysb columns are written.
  - Conv diag weights built ON CHIP (DVE stride-0-broadcast
    tensor_tensor from a [128, 56] dgv input) instead of a 1.75MB dgw
    DMA; conv side-taps 7/8 moved DVE -> gpsimd; pad copies split
    ACT/DVE evenly.
  - kvsb conv-input buffers: only the SAME-pad border cells are
    memset (Pool) instead of the whole [128, 1089] plane; a separate
    [128, 512] zero strip feeds PE warm-up + PSUM zero-region matmuls.
  - Input DMAs coalesced/reordered by first-use (12 HWDGE dispatches).
"""
import sys

for _p in ("/opt/trn_rl_repo", "/opt/trn_rl_repo/concourse"):
    if _p not in sys.path:
        sys.path.insert(0, _p)

import numpy as np

import concourse.bass as bass
import concourse.mybir as mybir
import concourse.tile as tile
from concourse import bacc
from concourse.bass_utils import run_bass_kernel_spmd
from concourse.masks import make_identity

F32 = mybir.dt.float32
F16 = mybir.dt.float16
BF16 = mybir.dt.bfloat16
AF = mybir.ActivationFunctionType
ALU = mybir.AluOpType

DIM = 512
NH = 8
HD = 64
B, H, W = 2, 32, 32
L = H * W            # 1024 tokens per batch
KH = KW = 16
NS = KH * KW         # 256 conv-output spatial positions
M = NS * NH          # 2048 shared-KV positions
CH = 2 * DIM         # 1024 kv channels
SCALE = HD ** -0.5   # 0.125
PADW = 33            # padded conv input row (32 + 1 SAME pad after)
NPAD = PADW * PADW   # 1089

_NC_CACHE = {}
LABELS = {}


def _tag(bi, label):
    try:
        LABELS[bi.ins.name] = label
    except Exception:
        pass
    return bi


def _build_program():
    nc = bacc.Bacc(None)

    xT_d = nc.dram_tensor("xT", [DIM, L], F16, kind="ExternalInput")
    xTc_d = nc.dram_tensor("xTc", [DIM, 256], F16, kind="ExternalInput")
    wkv_d = nc.dram_tensor("wkv", [DIM, CH], F16, kind="ExternalInput")
    wq_d = nc.dram_tensor("wq", [DIM, DIM], F16, kind="ExternalInput")
    wo_d = nc.dram_tensor("wo", [DIM, DIM], F16, kind="ExternalInput")
    bpl_d = nc.dram_tensor("bpl", [CH, NS], F16, kind="ExternalInput")
    dgv_d = nc.dram_tensor("dgv", [128, 56], F32, kind="ExternalInput")
    # cst cols: 0-3 bq tiles, 4-7 bo tiles, 8+ conv tap scalars
    cst_d = nc.dram_tensor("cst", [128, 40], F32, kind="ExternalInput")
    y_d = nc.dram_tensor("y", [DIM, 256], F32, kind="ExternalOutput")
    import os as _os
    _dbg = _os.environ.get("BASSDBG") == "1"
    if _dbg:
        kT2_o = nc.dram_tensor("kT2o", [64, M], F16, kind="ExternalOutput")
        qT2_o = nc.dram_tensor("qT2o", [64, M], F16, kind="ExternalOutput")
        vaug_o = nc.dram_tensor("vaugo", [128, 16 * (HD + 1)], BF16,
                                kind="ExternalOutput")
        asb_o = nc.dram_tensor("asbo", [128, 16 * HD], F16,
                               kind="ExternalOutput")

    with tile.TileContext(nc) as tc:
        with tc.tile_pool(name="wp", bufs=1) as wp, \
             tc.tile_pool(name="expp", bufs=8) as expp, \
             tc.tile_pool(name="tmpq", bufs=2) as tmppool, \
             tc.tile_pool(name="ps", bufs=1, space="PSUM") as ps:

            # ---------------- input DMAs (priority order) ----------------
            cst = wp.tile([128, 40], F32, tag="cst")
            xTc = wp.tile([128, 4, 256], F16, tag="xTc")
            xTc_r = xTc_d[:, :].rearrange("(k p) t -> p k t", p=128)
            wq = wp.tile([128, 4, DIM], F16, tag="wq")
            wq_r = wq_d[:, :].rearrange("(k p) c -> p k c", p=128)
            xT = wp.tile([128, 4, L], F16, tag="xT")
            wkv = wp.tile([128, 4, CH], F16, tag="wkv")
            dgv = wp.tile([128, 56], F32, tag="dgv")
            bpl = wp.tile([128, 8, NS], F16, tag="bpl")
            wo = wp.tile([128, 4, DIM], F16, tag="wo")
            xT_r = xT_d[:, :].rearrange("(k p) t -> p k t", p=128)
            wkv_r = wkv_d[:, :].rearrange("(k p) c -> p k c", p=128)
            bpl_r = bpl_d[:, :].rearrange("(t p) s -> p t s", p=128)

            # critical-path order: K0/V0 proj operands, q operands, then
            # second-half tokens, conv bias plane, remaining kv weights,
            # y-proj weights last.
            nc.sync.dma_start(out=wkv[:, :, 0:256], in_=wkv_r[:, :, 0:256])
            nc.sync.dma_start(out=dgv, in_=dgv_d[:, :])
            nc.sync.dma_start(out=xT[:, 0:2, 0:512], in_=xT_r[:, 0:2, 0:512])
            nc.sync.dma_start(out=xT[:, 2:4, 0:512], in_=xT_r[:, 2:4, 0:512])
            nc.sync.dma_start(out=xT[:, :, 512:L], in_=xT_r[:, :, 512:L])
            nc.sync.dma_start(out=bpl[:, 0:2, :], in_=bpl_r[:, 0:2, :])
            nc.sync.dma_start(out=xTc, in_=xTc_r)
            nc.sync.dma_start(out=wq, in_=wq_r)
            nc.sync.dma_start(out=cst, in_=cst_d[:, :])
            nc.sync.dma_start(out=bpl[:, 2:8, :], in_=bpl_r[:, 2:8, :])
            nc.sync.dma_start(out=wkv[:, :, 256:512],
                              in_=wkv_r[:, :, 256:512])
            nc.sync.dma_start(out=wkv[:, :, 512:CH],
                              in_=wkv_r[:, :, 512:CH])
            nc.sync.dma_start(out=wo,
                              in_=wo_d[:, :].rearrange("(k p) c -> p k c", p=128))

            # ---------------- persistent SBUF state ----------------
            ident = wp.tile([128, 128], F16, tag="ident")
            make_identity(nc, ident)
            # preload exp ACT table during the DMA window
            warm = wp.tile([1, 1], F32, tag="warm")
            nc.vector.memset(warm, 0.0)
            nc.scalar.activation(warm[:, :], warm[:, :], AF.Exp)

            kT2 = wp.tile([64, M], F16, tag="kT2")
            qT2 = wp.tile([64, M], F16, tag="qT2")
            vaug = wp.tile([128, 16, HD + 1], BF16, tag="vaug")
            nc.vector.memset(vaug[:, :, HD:HD + 1], 1.0)
            attnT = [wp.tile([128, 256], F16, tag=f"attnT{k}",
                              name=f"attnT{k}") for k in range(4)]
            attn_sb = [wp.tile([128, 4, HD], F16, tag=f"attn_sb{k}",
                                name=f"attn_sb{k}") for k in range(4)]
            zr = [wp.tile([128, 4], F32, tag=f"zr{k}", name=f"zr{k}")
                  for k in range(4)]
            vacc = wp.tile([128, NS], F16, tag="vacc")
            ysb = [wp.tile([128, 2, 256], F32, tag=f"ysb{h}",
                          name=f"ysb{h}") for h in range(2)]

            # zero strip for PE warm-up and PSUM zero-region matmuls
            zstrip = wp.tile([128, 512], F16, tag="zstrip")
            nc.gpsimd.memset(zstrip[:, :], 0.0)

            # conv-input buffers: only SAME-pad border cells are zeroed
            # (col 32 of rows 0-32, then row 32 cols 0-31); the interior
            # is fully overwritten by pad_copy each use.
            kvsb = []
            for i in range(2):
                kb = wp.tile([128, NPAD], F16, tag=f"kvsb{i}",
                             name=f"kvsb{i}")
                colpad = bass.AP(tensor=kb.tensor, offset=kb.offset + 32,
                                 ap=[kb.ap[0], [PADW, PADW]])
                rowpad = bass.AP(tensor=kb.tensor,
                                 offset=kb.offset + PADW * 32,
                                 ap=[kb.ap[0], [1, 32]])
                nc.gpsimd.memset(colpad, 0.0)
                nc.gpsimd.memset(rowpad, 0.0)
                kvsb.append(kb)

            # conv diag weights built on-chip: dgt[:, dg*7+tap, :] =
            # diag(dgv[:, dg*7+tap]) via stride-0-broadcast tensor_tensor
            dgt = wp.tile([128, 56, 128], F16, tag="dgt")

            def diag_op(dg, eng=None):
                if eng is nc.scalar:
                    for tap in range(7):
                        i = dg * 7 + tap
                        _tag(nc.scalar.mul(dgt[:, i, :], ident[:, :],
                                           dgv[:, i:i + 1]), f"diag({dg})")
                    return
                id_b = bass.AP(tensor=ident.tensor, offset=ident.offset,
                               ap=[ident.ap[0], [0, 7], [1, 128]])
                dg_b = bass.AP(tensor=dgv.tensor, offset=dgv.offset + 7 * dg,
                               ap=[dgv.ap[0], [1, 7], [0, 128]])
                _tag((eng or nc.gpsimd).tensor_tensor(
                    dgt[:, dg * 7:(dg + 1) * 7, :],
                    id_b, dg_b, op=ALU.mult), f"diag({dg})")

            diag_op(0, eng=nc.vector)
            diag_op(1, eng=nc.vector)
            diag_op(4)
            diag_op(5)

            # PE warm-up: keep the array busy through the DMA window so the
            # p-state ramp completes before the real matmuls arrive.
            for _w in range(4):
                wmm = ps.tile([128, 512], F32, tag="kvp", bufs=1)
                nc.tensor.matmul(wmm[:, :], ident[:, :], zstrip[:, :],
                                 start=True, stop=True)

            # ---------------- helpers ----------------
            def pad_copy(dg, nh, kvp, eng):
                kb = kvsb[dg % 2]
                dst = bass.AP(tensor=kb.tensor,
                              offset=kb.offset + PADW * 16 * nh,
                              ap=[kb.ap[0], [PADW, 16], [1, 32]])
                src = kvp[:, :].rearrange("p (a b) -> p a b", b=32)
                if eng is nc.scalar:
                    _tag(eng.copy(dst, src), f"pad({dg},{nh})")
                else:
                    _tag(eng.tensor_copy(dst, src), f"pad({dg},{nh})")

            tmpp = tmppool

            def conv(dg):
                """Conv taps 0-6 on PE (PSUM cvp); taps 7,8 + bias into an
                SBUF side-accumulator on gpsimd, merged at the K/V stt."""
                kb = kvsb[dg % 2]
                tmp = tmpp.tile([128, NS], F32, tag="tmp")
                for tap in (7, 8):
                    dy, dx = tap // 3, tap % 3
                    win = bass.AP(tensor=kb.tensor,
                                  offset=kb.offset + PADW * dy + dx,
                                  ap=[kb.ap[0], [2 * PADW, KH], [2, KW]])
                    col = 8 + (tap - 5) * 8 + dg
                    in1 = bpl[:, dg, :] if tap == 7 else tmp[:, :]
                    _tag(nc.vector.scalar_tensor_tensor(
                        tmp[:, :], win, cst[:, col:col + 1], in1,
                        op0=ALU.mult, op1=ALU.add), f"ctap({dg},{tap})")
                cvp = ps.tile([128, NS], F32, tag="kvp", bufs=1)
                for tap in range(7):
                    dy, dx = tap // 3, tap % 3
                    win = bass.AP(tensor=kb.tensor,
                                  offset=kb.offset + PADW * dy + dx,
                                  ap=[kb.ap[0], [2 * PADW, KH], [2, KW]])
                    _tag(nc.tensor.matmul(cvp[:, :], dgt[:, dg * 7 + tap, :], win,
                                     start=(tap == 0), stop=(tap == 6)),
                         f"cmm({dg},{tap})")
                return cvp, tmp

            def k_tile(t, cvp, tmp):
                # gpsimd cannot read PSUM: merge to SBUF, then cross-copy
                kacc = tmpp.tile([128, NS], F16, tag="kacc")
                _tag(nc.vector.scalar_tensor_tensor(
                    kacc[:, :], cvp[:, :], 1.0, tmp[:, :],
                    op0=ALU.mult, op1=ALU.add), f"kmerge({t})")
                _tag(nc.gpsimd.tensor_copy(kT2[:, (2 * t) * 256:(2 * t + 1) * 256],
                                      kacc[0:64, :]), f"kcopy({t},0)")
                _tag(nc.gpsimd.tensor_copy(
                    kT2[:, (2 * t + 1) * 256:(2 * t + 2) * 256],
                    kacc[64:128, :]), f"kcopy({t},1)")

            def v_tile(t, cvp, tmp):
                _tag(nc.vector.scalar_tensor_tensor(
                    vacc[:, :], cvp[:, :], 1.0, tmp[:, :],
                    op0=ALU.mult, op1=ALU.add), f"vmerge({t})")
                for gi in range(2):
                    vt = ps.tile([128, 128], F16, tag="kvp", bufs=1)
                    for sh in range(2):
                        nc.tensor.transpose(
                            vt[:, sh * 64:(sh + 1) * 64],
                            vacc[gi * 64:(gi + 1) * 64,
                                 sh * 128:(sh + 1) * 128],
                            ident[gi * 64:(gi + 1) * 64,
                                  gi * 64:(gi + 1) * 64])
                    mt0 = 4 * t + 2 * gi
                    _tag(nc.vector.tensor_copy(
                        vaug[:, mt0:mt0 + 2, 0:HD],
                        vt[:, :].rearrange("p (a b) -> p a b", b=64)),
                         f"vcopy({t},{gi})")

            # per-uq AV accumulators, one PSUM bank each so normalize
            # reads of uq never serialize against av writes of uq+1
            avt = [ps.tile([128, 4, 128], F32, tag="av", bufs=4,
                           name=f"av{u}") for u in range(4)]
            for u in range(4):
                zmm = bass.AP(tensor=avt[u].tensor, offset=avt[u].offset,
                              ap=[avt[u].ap[0], [1, 512]])
                nc.tensor.matmul(zmm, ident[:, :], zstrip[:, :],
                                 start=True, stop=True, skip_group_check=True)

            ucount = [0]

            def sc_unit(mt, uq, force_eng=None):
                st = ps.tile([128, 512], F32, tag="st", bufs=3)
                _tag(nc.tensor.matmul(st[:, :],
                                 kT2[:, mt * 128:(mt + 1) * 128],
                                 qT2[:, uq * 512:(uq + 1) * 512],
                                 start=True, stop=True), f"sc({mt},{uq})")
                ucount[0] += 1
                dve = ucount[0] % 3 == 0
                if force_eng is not None:
                    dve = force_eng == "dve"
                if dve:
                    # Schraudolph exp on DVE: bf16 bits = s*scale*184.66 +
                    # (16256 - 7.22); int16 convert, bitcast view as bf16.
                    exd = expp.tile([128, 512], mybir.dt.int16, tag="ex",
                                    name="exd")
                    _tag(nc.vector.tensor_scalar(
                        exd[:, :], st[:, :], 23.08294, 16248.78,
                        op0=ALU.mult, op1=ALU.add), f"expD({mt},{uq})")
                    return exd.bitcast(BF16)
                ex = expp.tile([128, 512], BF16, tag="ex")
                _tag(nc.scalar.activation(ex[:, :], st[:, :], AF.Exp,
                                     scale=float(SCALE)), f"expA({mt},{uq})")
                return ex

            def av_unit(mt, uq, ex):
                for j in range(4):
                    _tag(nc.tensor.matmul(avt[uq][:, j, 0:HD + 1],
                                     ex[:, j * 128:(j + 1) * 128],
                                     vaug[:, mt, :],
                                     start=False, stop=(mt == 15),
                                     skip_group_check=True), f"av({mt},{uq},{j})")

            # ---------------- q projection ----------------
            def emit_qproj(t):
                qp = ps.tile([128, 512], F32, tag="st", bufs=3)
                qpv = qp[:, 0:256]
                for k in range(4):
                    _tag(nc.tensor.matmul(qpv, wq[:, k, t * 128:(t + 1) * 128],
                                     xTc[:, k, :],
                                     start=(k == 0), stop=(k == 3)), f"qp({t},{k})")
                nc.vector.tensor_scalar_add(qpv, qpv, cst[:, t:t + 1])
                nc.vector.tensor_copy(qT2[:, (2 * t) * 256:(2 * t + 1) * 256],
                                      qp[0:64, 0:256])
                qstage = tmpp.tile([128, 256], F16, tag="kacc", name="qstage")
                nc.vector.tensor_copy(qstage[64:128, :], qp[64:128, 0:256])
                nc.gpsimd.tensor_copy(qT2[:, (2 * t + 1) * 256:(2 * t + 2) * 256],
                                      qstage[64:128, :])

            # ---------------- fused P1/P2 pipeline ----------------
            kvp_store = {}

            def pad_eng(dg):
                # gpsimd cannot access PSUM; split PSUM reads DVE/ACT
                if dg % 2 == 0:
                    return nc.scalar
                return nc.vector

            def proj_op(dg, nh):
                if dg % 2 == 0:  # K tile -> st slots
                    kvp = ps.tile([128, 512], F32, tag="st", bufs=3,
                                  name="kvpk")
                else:
                    kvp = ps.tile([128, 512], F32, tag="kvp", bufs=1)
                for k in range(4):
                    _tag(nc.tensor.matmul(kvp[:, :],
                                     wkv[:, k, dg * 128:(dg + 1) * 128],
                                     xT[:, k, nh * 512:(nh + 1) * 512],
                                     start=(k == 0), stop=(k == 3)),
                         f"proj({dg},{nh},{k})")
                kvp_store[(dg, nh)] = kvp

            def pad_op(dg, nh):
                pad_copy(dg, nh, kvp_store.pop((dg, nh)), pad_eng(dg))

            def p1_ops(t):
                dgk, dgv_ = 2 * t, 2 * t + 1
                mk = lambda f, *a: (lambda: f(*a))
                cstore = {}

                def conv_op(dg):
                    cstore[dg] = conv(dg)

                def ktail(tt):
                    k_tile(tt, *cstore.pop(2 * tt))

                def vtail(tt):
                    v_tile(tt, *cstore.pop(2 * tt + 1))

                ops = [
                    mk(proj_op, dgk, 0), mk(pad_op, dgk, 0),
                    mk(proj_op, dgv_, 0), mk(pad_op, dgv_, 0),
                    mk(proj_op, dgk, 1), mk(pad_op, dgk, 1),
                    mk(proj_op, dgv_, 1), mk(pad_op, dgv_, 1),
                    mk(conv_op, dgk), mk(ktail, t),
                    mk(conv_op, dgv_), mk(vtail, t),
                ]
                return ops

            ops0 = p1_ops(0)
            for op in ops0[0:8]:     # K0/V0 both halves
                op()
            diag_op(6, eng=nc.scalar)
            diag_op(7, eng=nc.scalar)
            ops0[8]()                # conv K0
            emit_qproj(0)
            emit_qproj(1)
            ops0[9]()                # ktail(0)
            ops0[10]()               # conv V0
            emit_qproj(2)
            emit_qproj(3)
            ops0[11]()               # vtail(0)
            diag_op(2, eng=nc.vector)
            diag_op(3, eng=nc.vector)

            # ---------------- normalize / attnT helpers ----------------
            def slot_of(chunk):
                h, lh = chunk // 2, chunk % 2
                return 4 * (h // 2) + 2 * lh + (h % 2)

            def norm_chunk(chunk):
                slot = slot_of(chunk)
                kk, sl = slot // 4, slot % 4
                uq, j = chunk // 4, chunk % 4
                src_v = avt[uq][:, j, 0:HD]
                src_z = avt[uq][:, j, HD:HD + 1]
                if chunk % 2 == 0:
                    _tag(nc.vector.reciprocal(zr[uq][:, j:j + 1], src_z),
                         f"recip({chunk})")
                    _tag(nc.vector.tensor_scalar_mul(
                        attn_sb[kk][:, sl, :], src_v,
                        zr[uq][:, j:j + 1]), f"nmul({chunk})")
                else:
                    _tag(nc.vector.reciprocal(zr[uq][:, j:j + 1], src_z),
                         f"recip({chunk})")
                    _tag(nc.scalar.mul(attn_sb[kk][:, sl, :], src_v,
                                  zr[uq][:, j:j + 1]), f"nmul({chunk})")

            vt2a_pre = {}

            def attnT_block(kk):
                # transposes + SBUF staging only (y matmuls emitted later)
                for lh in range(2):
                    if lh == 0:
                        vt2 = ps.tile([128, 128], F16, tag="kvp", bufs=1)
                    elif kk in vt2a_pre:
                        vt2 = vt2a_pre.pop(kk)
                    else:
                        vt2 = ps.tile([128, 128], F16, tag="av", bufs=4,
                                      name="vt2a")
                    asb = attn_sb[kk]
                    src_ap = bass.AP(
                        tensor=asb.tensor,
                        offset=asb.offset + 2 * lh * HD,
                        ap=[asb.ap[0], [1, 128]])
                    _tag(nc.tensor.transpose(vt2[:, :], src_ap, ident[:, :]),
                         f"atT({kk},{lh})")
                    if lh == 0:
                        _tag(nc.scalar.copy(
                            attnT[kk][:, 0:128], vt2[:, :]), f"atC({kk},{lh})")
                    else:
                        _tag(nc.vector.tensor_copy(
                            attnT[kk][:, 128:256], vt2[:, :]), f"atC({kk},{lh})")

            # Unit stream with av lagging sc by one; p1 micro-ops of tile
            # t+1 round-robined between tile t's units.  Groups t=0..2 run
            # mt-major; the last group runs uq-major so each uq's
            # normalize/attnT pipeline overlaps the next uq's score units.
            pending = []

            def drain_pending(keep):
                while len(pending) > keep:
                    av_unit(*pending.pop(0))

            for t in range(3):
                us = [(mt, uq) for mt in range(4 * t, 4 * t + 4)
                      for uq in range(4)]
                chunks = p1_ops(t + 1)
                nu, nch = len(us), len(chunks)
                ci = 0
                for i, u in enumerate(us):
                    mt, uq = u
                    ex = sc_unit(mt, uq)
                    pending.append((mt, uq, ex))
                    drain_pending(2)
                    want = ((i + 1) * nch) // nu
                    while ci < want:
                        chunks[ci]()
                        ci += 1

            # Last group: uq-major score stream with exps strictly
            # alternating engines; uq's normalize + attnT emitted AFTER
            # uq+1's score units so the exp cadence stays hot.
            for uq in range(4):
                for i, mt in enumerate(range(12, 16)):
                    force = "dve" if (uq * 4 + i) % 2 else "act"
                    ex = sc_unit(mt, uq, force_eng=force)
                    pending.append((mt, uq, ex))
                    drain_pending(2)
                if uq >= 1:
                    for j in range(4):
                        norm_chunk((uq - 1) * 4 + j)
                    attnT_block(uq - 1)
            drain_pending(0)
            # uq3 normalize must be emitted before its av slot is recycled
            for j in range(4):
                norm_chunk(12 + j)
            # vt2a(3) takes av slot 3 (WAR = uq3 norm reads, its true dep);
            # yp0-2 take the st slots freed by the last score units; yp3
            # takes av slot 0 (WAR = atC(0,1), long emitted).
            vt2a_pre[3] = ps.tile([128, 128], F16, tag="av", bufs=4,
                                  name="vt2a")
            yps = [ps.tile([128, 256], F32, tag="st", bufs=3,
                           name=f"yp{m}") for m in range(3)]
            yps.append(ps.tile([128, 256], F32, tag="av", bufs=4,
                               name="yp3"))

            def y_block(kk, with_copies=False):
                for m in range(4):
                    _tag(nc.tensor.matmul(
                        yps[m][:, :],
                        wo[:, kk, m * 128:(m + 1) * 128],
                        attnT[kk][:, :],
                        start=(kk == 0), stop=(kk == 3),
                        skip_group_check=True), f"y({kk},{m})")
                    if with_copies:
                        if m % 2 == 0:
                            _tag(nc.vector.tensor_scalar_add(
                                ysb[m // 2][:, m % 2, :], yps[m][:, :],
                                cst[:, 4 + m:5 + m]), f"ysb({m})")
                        else:
                            _tag(nc.scalar.add(
                                ysb[m // 2][:, m % 2, :], yps[m][:, :],
                                cst[:, 4 + m:5 + m]), f"ysb({m})")
                        if m == 1:
                            nc.sync.dma_start(
                                out=y_d[0:256, :].rearrange(
                                    "(m p) t -> p m t", p=128),
                                in_=ysb[0][:, :, :])

            for kk in range(3):
                y_block(kk)
            attnT_block(3)
            y_block(3, with_copies=True)
            if _dbg:
                nc.sync.dma_start(out=kT2_o[:, :], in_=kT2[:, :])
                nc.sync.dma_start(out=qT2_o[:, :], in_=qT2[:, :])
                nc.sync.dma_start(out=vaug_o[:, :],
                                  in_=vaug[:, :, :].rearrange("p a b -> p (a b)"))
                nc.sync.dma_start(out=asb_o[:, :],
                                  in_=attn_sb[:, :, :].rearrange("p a b -> p (a b)"))
            nc.gpsimd.dma_start(
                out=y_d[256:512, :].rearrange("(m p) t -> p m t", p=128),
                in_=ysb[1][:, :, :])

    nc.finalize()
    return nc


def _get_program():
    if "nc" not in _NC_CACHE:
        _NC_CACHE["nc"] = _build_program()
    return _NC_CACHE["nc"]


def _host_prep(x, wq, bq, wkv, bkv, dw_kernel, dw_bias, wo, bo):
    """Build the 8 per-core input maps."""
    x = np.ascontiguousarray(np.asarray(x, np.float32))
    wq16 = np.asarray(wq, np.float32).astype(np.float16)
    wo16 = np.asarray(wo, np.float32).astype(np.float16)
    bq = np.asarray(bq, np.float32)
    bkv = np.asarray(bkv, np.float32)
    dw_bias = np.asarray(dw_bias, np.float32)
    bo = np.asarray(bo, np.float32)
    dww = np.asarray(dw_kernel, np.float32).reshape(9, CH).T.copy()  # [1024, 9]

    # channel-tile processing order K0 V0 K1 V1 ... ; dg -> channel base
    dg_base = []
    for t in range(4):
        dg_base += [t * 128, DIM + t * 128]

    # wkv columns reordered to dg order
    wkv_f = np.asarray(wkv, np.float32)
    wkv16 = np.empty((DIM, CH), np.float16)
    for dg in range(8):
        b0 = dg_base[dg]
        wkv16[:, dg * 128:(dg + 1) * 128] = wkv_f[:, b0:b0 + 128]

    # bias plane: dw_bias + bkv * sum(valid taps), SAME padding aware,
    # rows in dg order
    oy = np.arange(KH)
    valid_y = (2 * oy[:, None] + np.arange(3)[None, :]) < H      # [16, 3]
    valid_x = valid_y.copy()
    wsum = np.zeros((CH, KH, KW), np.float32)
    for tap in range(9):
        dy, dx = tap // 3, tap % 3
        m2 = np.outer(valid_y[:, dy], valid_x[:, dx]).astype(np.float32)
        wsum += dww[:, tap][:, None, None] * m2[None, :, :]
    bpl_full = (dw_bias[:, None] + bkv[:, None] * wsum.reshape(CH, NS))
    bpl16 = np.empty((CH, NS), np.float16)
    for dg in range(8):
        b0 = dg_base[dg]
        bpl16[dg * 128:(dg + 1) * 128] = bpl_full[b0:b0 + 128]

    # conv diag values [128, 56] f32, dg-major then tap (taps 0-6 on PE)
    dgv = np.zeros((128, 56), np.float32)
    for dg in range(8):
        b0 = dg_base[dg]
        for tap in range(7):
            dgv[:, dg * 7 + tap] = dww[b0:b0 + 128, tap]

    cst = np.zeros((128, 40), np.float32)
    cst[:, 0:4] = bq.reshape(4, 128).T
    cst[:, 4:8] = bo.reshape(4, 128).T
    for dg in range(8):
        b0 = dg_base[dg]
        for tap in range(5, 9):
            cst[:, 8 + (tap - 5) * 8 + dg] = dww[b0:b0 + 128, tap]

    in_maps = []
    for c in range(8):
        b, j = c // 4, c % 4
        xT = x[b].reshape(L, DIM).T.astype(np.float16)
        in_maps.append({
            "xT": np.ascontiguousarray(xT),
            "xTc": np.ascontiguousarray(xT[:, j * 256:(j + 1) * 256]),
            "wkv": wkv16, "wq": wq16, "wo": wo16,
            "bpl": bpl16, "dgv": dgv, "cst": cst,
        })
    return in_maps


def kernel(**inputs) -> np.ndarray:
    nc = _get_program()
    in_maps = _host_prep(**inputs)
    res = run_bass_kernel_spmd(nc, in_maps, core_ids=list(range(8)))
    out = np.zeros((B, H, W, DIM), np.float32)
    flat = out.reshape(B, L, DIM)
    for c in range(8):
        b, j = c // 4, c % 4
        flat[b, j * 256:(j + 1) * 256, :] = res.results[c]["y"].T
    return out


# revision 17
# speedup vs baseline: 1.0572x; 1.0220x over previous
"""MobileMQA Trainium2 kernel v3 (8 NeuronCores, SPMD).

Reference computation (per batch b of 2):
  q  = x @ wq + bq                         [1024 tok, 512]
  kv = x @ wkv + bkv                       [1024 tok, 1024]
  kv = depthwise3x3_s2_same(kv) + dw_bias  [256 sp, 1024]
  k, v = split(kv)  -> shared-KV length M=2048 (channel fold)
  attn = softmax(q @ k^T * 0.125); out = attn @ v
  y = out @ wo + bo

Sharding: core c handles batch b=c//4, query chunk j=c%4 (256 tokens).
KV path (proj+conv) replicated across the 4 cores of a batch (MQA).

v3 design vs v2 (67765 ns):
  - Tail restructured: last mt-group runs uq-major; per-uq normalize +
    attnT transposes pipeline INTO the attention stream (prev-uq blocks
    emitted between next-uq score units). y-proj is a tight kk-major
    burst at the end; bo folds into the final PSUM->SBUF copies via cst
    (bias matmuls + ones16/bo16 inputs dropped); per-half output DMAs
    issue as soon as their ysb columns are written.
  - Conv diag weights built ON CHIP (DVE stride-0-broadcast
    tensor_tensor from a [128, 56] dgv input) instead of a 1.75MB dgw
    DMA; conv side-taps 7/8 moved DVE -> gpsimd; pad copies split
    ACT/DVE evenly.
  - kvsb conv-input buffers: only the SAME-pad border cells are
    memset (Pool) instead of the whole [128, 1089] plane; a separate
    [128, 512] zero strip feeds PE warm-up + PSUM zero-region matmuls.
  - Input DMAs coalesced/reordered by first-use (12 HWDGE dispatches).
"""
import sys

for _p in ("/opt/trn_rl_repo", "/opt/trn_rl_repo/concourse"):
    if _p not in sys.path:
        sys.path.insert(0, _p)

import numpy as np

import concourse.bass as bass
import concourse.mybir as mybir
import concourse.tile as tile
from concourse import bacc
from concourse.bass_utils import run_bass_kernel_spmd
from concourse.masks import make_identity

F32 = mybir.dt.float32
F16 = mybir.dt.float16
BF16 = mybir.dt.bfloat16
AF = mybir.ActivationFunctionType
ALU = mybir.AluOpType

DIM = 512
NH = 8
HD = 64
B, H, W = 2, 32, 32
L = H * W            # 1024 tokens per batch
KH = KW = 16
NS = KH * KW         # 256 conv-output spatial positions
M = NS * NH          # 2048 shared-KV positions
CH = 2 * DIM         # 1024 kv channels
SCALE = HD ** -0.5   # 0.125
PADW = 33            # padded conv input row (32 + 1 SAME pad after)
NPAD = PADW * PADW   # 1089

_NC_CACHE = {}
LABELS = {}


def _tag(bi, label):
    try:
        LABELS[bi.ins.name] = label
    except Exception:
        pass
    return bi


def _build_program():
    nc = bacc.Bacc(None)

    F8 = mybir.dt.float8e4
    xTh_d = nc.dram_tensor("xTh", [DIM, L], F8, kind="ExternalInput")
    xTl_d = nc.dram_tensor("xTl", [DIM, L], F8, kind="ExternalInput")
    xTch_d = nc.dram_tensor("xTch", [DIM, 256], F8, kind="ExternalInput")
    xTcl_d = nc.dram_tensor("xTcl", [DIM, 256], F8, kind="ExternalInput")
    wkvh_d = nc.dram_tensor("wkvh", [DIM, CH], F8, kind="ExternalInput")
    wkvl_d = nc.dram_tensor("wkvl", [DIM, CH], F8, kind="ExternalInput")
    wqh_d = nc.dram_tensor("wqh", [DIM, DIM], F8, kind="ExternalInput")
    wql_d = nc.dram_tensor("wql", [DIM, DIM], F8, kind="ExternalInput")
    wo_d = nc.dram_tensor("wo", [DIM, DIM], F16, kind="ExternalInput")
    bpl_d = nc.dram_tensor("bpl", [CH, NS], F16, kind="ExternalInput")
    dgv_d = nc.dram_tensor("dgv", [128, 56], F32, kind="ExternalInput")
    # cst cols: 0-3 bq tiles, 4-7 bo tiles, 8+ conv tap scalars
    cst_d = nc.dram_tensor("cst", [128, 40], F32, kind="ExternalInput")
    y_d = nc.dram_tensor("y", [DIM, 256], F32, kind="ExternalOutput")
    import os as _os
    _dbg = _os.environ.get("BASSDBG") == "1"
    if _dbg:
        kT2_o = nc.dram_tensor("kT2o", [64, M], F16, kind="ExternalOutput")
        qT2_o = nc.dram_tensor("qT2o", [64, M], F16, kind="ExternalOutput")
        vaug_o = nc.dram_tensor("vaugo", [128, 16 * (HD + 1)], BF16,
                                kind="ExternalOutput")
        asb_o = nc.dram_tensor("asbo", [128, 16 * HD], F16,
                               kind="ExternalOutput")

    with tile.TileContext(nc) as tc:
        with tc.tile_pool(name="wp", bufs=1) as wp, \
             tc.tile_pool(name="expp", bufs=8) as expp, \
             tc.tile_pool(name="tmpq", bufs=2) as tmppool, \
             tc.tile_pool(name="ps", bufs=1, space="PSUM") as ps:

            # ---------------- input DMAs (priority order) ----------------
            cst = wp.tile([128, 40], F32, tag="cst")
            rr = lambda d: d[:, :].rearrange("(k p) t -> p k t", p=128)
            xTc = [wp.tile([128, 4, 256], F8, tag=f"xTc{h}", name=f"xTc{h}")
                   for h in range(2)]
            wq = [wp.tile([128, 4, DIM], F8, tag=f"wq{h}", name=f"wq{h}")
                  for h in range(2)]
            xT = [wp.tile([128, 4, L], F8, tag=f"xT{h}", name=f"xT{h}")
                  for h in range(2)]
            wkv = [wp.tile([128, 4, CH], F8, tag=f"wkv{h}", name=f"wkv{h}")
                   for h in range(2)]
            dgv = wp.tile([128, 56], F32, tag="dgv")
            bpl = wp.tile([128, 8, NS], F16, tag="bpl")
            wo = wp.tile([128, 4, DIM], F16, tag="wo")
            xT_r = [rr(xTh_d), rr(xTl_d)]
            wkv_r = [rr(wkvh_d), rr(wkvl_d)]
            xTc_r = [rr(xTch_d), rr(xTcl_d)]
            wq_r = [rr(wqh_d), rr(wql_d)]
            bpl_r = bpl_d[:, :].rearrange("(t p) s -> p t s", p=128)

            # critical-path order: K0/V0 proj operands, q operands, then
            # second-half tokens, conv bias plane, remaining kv weights,
            # y-proj weights last.
            for h in range(2):
                nc.sync.dma_start(out=wkv[h][:, :, 0:256],
                                  in_=wkv_r[h][:, :, 0:256])
            nc.sync.dma_start(out=dgv, in_=dgv_d[:, :])
            for h in range(2):
                nc.sync.dma_start(out=xT[h][:, :, 0:512],
                                  in_=xT_r[h][:, :, 0:512])
            for h in range(2):
                nc.sync.dma_start(out=xT[h][:, :, 512:L],
                                  in_=xT_r[h][:, :, 512:L])
            nc.sync.dma_start(out=bpl[:, 0:2, :], in_=bpl_r[:, 0:2, :])
            for h in range(2):
                nc.sync.dma_start(out=xTc[h], in_=xTc_r[h])
            for h in range(2):
                nc.sync.dma_start(out=wq[h], in_=wq_r[h])
            nc.sync.dma_start(out=cst, in_=cst_d[:, :])
            nc.sync.dma_start(out=bpl[:, 2:8, :], in_=bpl_r[:, 2:8, :])
            for h in range(2):
                nc.sync.dma_start(out=wkv[h][:, :, 256:CH],
                                  in_=wkv_r[h][:, :, 256:CH])
            nc.sync.dma_start(out=wo,
                              in_=wo_d[:, :].rearrange("(k p) c -> p k c", p=128))

            # ---------------- persistent SBUF state ----------------
            ident = wp.tile([128, 128], F16, tag="ident")
            make_identity(nc, ident)
            # preload exp ACT table during the DMA window
            warm = wp.tile([1, 1], F32, tag="warm")
            nc.vector.memset(warm, 0.0)
            nc.scalar.activation(warm[:, :], warm[:, :], AF.Exp)

            kT2 = wp.tile([64, M], F16, tag="kT2")
            qT2 = wp.tile([64, M], F16, tag="qT2")
            vaug = wp.tile([128, 16, HD + 1], BF16, tag="vaug")
            nc.vector.memset(vaug[:, :, HD:HD + 1], 1.0)
            attnT = [wp.tile([128, 256], F16, tag=f"attnT{k}",
                              name=f"attnT{k}") for k in range(4)]
            attn_sb = [wp.tile([128, 4, HD], F16, tag=f"attn_sb{k}",
                                name=f"attn_sb{k}") for k in range(4)]
            zr = [wp.tile([128, 4], F32, tag=f"zr{k}", name=f"zr{k}")
                  for k in range(4)]
            vacc = wp.tile([128, NS], F16, tag="vacc")
            ysb = [wp.tile([128, 2, 256], F32, tag=f"ysb{h}",
                          name=f"ysb{h}") for h in range(2)]

            # zero strip for PE warm-up and PSUM zero-region matmuls
            zstrip = wp.tile([128, 512], F16, tag="zstrip")
            nc.gpsimd.memset(zstrip[:, :], 0.0)

            # conv-input buffers: only SAME-pad border cells are zeroed
            # (col 32 of rows 0-32, then row 32 cols 0-31); the interior
            # is fully overwritten by pad_copy each use.
            kvsb = []
            for i in range(2):
                kb = wp.tile([128, NPAD], F16, tag=f"kvsb{i}",
                             name=f"kvsb{i}")
                colpad = bass.AP(tensor=kb.tensor, offset=kb.offset + 32,
                                 ap=[kb.ap[0], [PADW, PADW]])
                rowpad = bass.AP(tensor=kb.tensor,
                                 offset=kb.offset + PADW * 32,
                                 ap=[kb.ap[0], [1, 32]])
                nc.gpsimd.memset(colpad, 0.0)
                nc.gpsimd.memset(rowpad, 0.0)
                kvsb.append(kb)

            # conv diag weights built on-chip: dgt[:, dg*7+tap, :] =
            # diag(dgv[:, dg*7+tap]) via stride-0-broadcast tensor_tensor
            dgt = wp.tile([128, 56, 128], F16, tag="dgt")

            def diag_op(dg, eng=None):
                if eng is nc.scalar:
                    for tap in range(7):
                        i = dg * 7 + tap
                        _tag(nc.scalar.mul(dgt[:, i, :], ident[:, :],
                                           dgv[:, i:i + 1]), f"diag({dg})")
                    return
                id_b = bass.AP(tensor=ident.tensor, offset=ident.offset,
                               ap=[ident.ap[0], [0, 7], [1, 128]])
                dg_b = bass.AP(tensor=dgv.tensor, offset=dgv.offset + 7 * dg,
                               ap=[dgv.ap[0], [1, 7], [0, 128]])
                _tag((eng or nc.gpsimd).tensor_tensor(
                    dgt[:, dg * 7:(dg + 1) * 7, :],
                    id_b, dg_b, op=ALU.mult), f"diag({dg})")

            diag_op(0, eng=nc.vector)
            diag_op(1, eng=nc.vector)
            diag_op(4)
            diag_op(5)

            # PE warm-up: keep the array busy through the DMA window so the
            # p-state ramp completes before the real matmuls arrive.
            for _w in range(4):
                wmm = ps.tile([128, 512], F32, tag="kvp", bufs=1)
                nc.tensor.matmul(wmm[:, :], ident[:, :], zstrip[:, :],
                                 start=True, stop=True)

            # ---------------- helpers ----------------
            def pad_copy(dg, nh, kvp, eng):
                kb = kvsb[dg % 2]
                dst = bass.AP(tensor=kb.tensor,
                              offset=kb.offset + PADW * 16 * nh,
                              ap=[kb.ap[0], [PADW, 16], [1, 32]])
                src = kvp[:, :].rearrange("p (a b) -> p a b", b=32)
                if eng is nc.scalar:
                    _tag(eng.mul(dst, src, 1.0 / 16.0), f"pad({dg},{nh})")
                else:
                    _tag(eng.tensor_scalar_mul(dst, src, 1.0 / 16.0),
                         f"pad({dg},{nh})")

            tmpp = tmppool

            def conv(dg):
                """Conv taps 0-6 on PE (PSUM cvp); taps 7,8 + bias into an
                SBUF side-accumulator on gpsimd, merged at the K/V stt."""
                kb = kvsb[dg % 2]
                tmp = tmpp.tile([128, NS], F32, tag="tmp")
                for tap in (7, 8):
                    dy, dx = tap // 3, tap % 3
                    win = bass.AP(tensor=kb.tensor,
                                  offset=kb.offset + PADW * dy + dx,
                                  ap=[kb.ap[0], [2 * PADW, KH], [2, KW]])
                    col = 8 + (tap - 5) * 8 + dg
                    in1 = bpl[:, dg, :] if tap == 7 else tmp[:, :]
                    _tag(nc.vector.scalar_tensor_tensor(
                        tmp[:, :], win, cst[:, col:col + 1], in1,
                        op0=ALU.mult, op1=ALU.add), f"ctap({dg},{tap})")
                cvp = ps.tile([128, NS], F32, tag="kvp", bufs=1)
                for tap in range(7):
                    dy, dx = tap // 3, tap % 3
                    win = bass.AP(tensor=kb.tensor,
                                  offset=kb.offset + PADW * dy + dx,
                                  ap=[kb.ap[0], [2 * PADW, KH], [2, KW]])
                    _tag(nc.tensor.matmul(cvp[:, :], dgt[:, dg * 7 + tap, :], win,
                                     start=(tap == 0), stop=(tap == 6)),
                         f"cmm({dg},{tap})")
                return cvp, tmp

            def k_tile(t, cvp, tmp):
                # gpsimd cannot read PSUM: merge to SBUF, then cross-copy
                kacc = tmpp.tile([128, NS], F16, tag="kacc")
                _tag(nc.vector.scalar_tensor_tensor(
                    kacc[:, :], cvp[:, :], 1.0, tmp[:, :],
                    op0=ALU.mult, op1=ALU.add), f"kmerge({t})")
                _tag(nc.gpsimd.tensor_copy(kT2[:, (2 * t) * 256:(2 * t + 1) * 256],
                                      kacc[0:64, :]), f"kcopy({t},0)")
                _tag(nc.gpsimd.tensor_copy(
                    kT2[:, (2 * t + 1) * 256:(2 * t + 2) * 256],
                    kacc[64:128, :]), f"kcopy({t},1)")

            def v_tile(t, cvp, tmp):
                _tag(nc.vector.scalar_tensor_tensor(
                    vacc[:, :], cvp[:, :], 1.0, tmp[:, :],
                    op0=ALU.mult, op1=ALU.add), f"vmerge({t})")
                for gi in range(2):
                    vt = ps.tile([128, 128], F16, tag="kvp", bufs=1)
                    for sh in range(2):
                        nc.tensor.transpose(
                            vt[:, sh * 64:(sh + 1) * 64],
                            vacc[gi * 64:(gi + 1) * 64,
                                 sh * 128:(sh + 1) * 128],
                            ident[gi * 64:(gi + 1) * 64,
                                  gi * 64:(gi + 1) * 64])
                    mt0 = 4 * t + 2 * gi
                    _tag(nc.vector.tensor_copy(
                        vaug[:, mt0:mt0 + 2, 0:HD],
                        vt[:, :].rearrange("p (a b) -> p a b", b=64)),
                         f"vcopy({t},{gi})")

            # per-uq AV accumulators, one PSUM bank each so normalize
            # reads of uq never serialize against av writes of uq+1
            avt = [ps.tile([128, 4, 128], F32, tag="av", bufs=4,
                           name=f"av{u}") for u in range(4)]
            for u in range(4):
                zmm = bass.AP(tensor=avt[u].tensor, offset=avt[u].offset,
                              ap=[avt[u].ap[0], [1, 512]])
                nc.tensor.matmul(zmm, ident[:, :], zstrip[:, :],
                                 start=True, stop=True, skip_group_check=True)

            ucount = [0]

            def sc_unit(mt, uq, force_eng=None):
                st = ps.tile([128, 512], F32, tag="st", bufs=3)
                _tag(nc.tensor.matmul(st[:, :],
                                 kT2[:, mt * 128:(mt + 1) * 128],
                                 qT2[:, uq * 512:(uq + 1) * 512],
                                 start=True, stop=True), f"sc({mt},{uq})")
                ucount[0] += 1
                dve = ucount[0] % 3 == 0
                if force_eng is not None:
                    dve = force_eng == "dve"
                if dve:
                    # Schraudolph exp on DVE: bf16 bits = s*scale*184.66 +
                    # (16256 - 7.22); int16 convert, bitcast view as bf16.
                    exd = expp.tile([128, 512], mybir.dt.int16, tag="ex",
                                    name="exd")
                    _tag(nc.vector.tensor_scalar(
                        exd[:, :], st[:, :], 23.08294, 16248.78,
                        op0=ALU.mult, op1=ALU.add), f"expD({mt},{uq})")
                    return exd.bitcast(BF16)
                ex = expp.tile([128, 512], BF16, tag="ex")
                _tag(nc.scalar.activation(ex[:, :], st[:, :], AF.Exp,
                                     scale=float(SCALE)), f"expA({mt},{uq})")
                return ex

            def av_unit(mt, uq, ex):
                for j in range(4):
                    _tag(nc.tensor.matmul(avt[uq][:, j, 0:HD + 1],
                                     ex[:, j * 128:(j + 1) * 128],
                                     vaug[:, mt, :],
                                     start=False, stop=(mt == 15),
                                     skip_group_check=True), f"av({mt},{uq},{j})")

            # ---------------- q projection ----------------
            def emit_qproj(t):
                qp = ps.tile([128, 512], F32, tag="st", bufs=3)
                qpv = qp[:, 0:256]
                first = True
                for s in range(2):
                    for wh, xh in ((0, 0), (0, 1), (1, 0)):
                        last = s == 1 and (wh, xh) == (1, 0)
                        _tag(nc.tensor.matmul(
                            qpv,
                            wq[wh][:, 2 * s:2 * s + 2,
                                   t * 128:(t + 1) * 128],
                            xTc[xh][:, 2 * s:2 * s + 2, :],
                            start=first, stop=last,
                            perf_mode=mybir.MatmulPerfMode.DoubleRow),
                             f"qp({t},{s})")
                        first = False
                nc.vector.tensor_scalar_add(qpv, qpv, cst[:, t:t + 1])
                nc.vector.tensor_scalar_mul(
                    qT2[:, (2 * t) * 256:(2 * t + 1) * 256],
                    qp[0:64, 0:256], 1.0 / 16.0)
                qstage = tmpp.tile([128, 256], F16, tag="kacc", name="qstage")
                nc.vector.tensor_scalar_mul(qstage[64:128, :],
                                            qp[64:128, 0:256], 1.0 / 16.0)
                nc.gpsimd.tensor_copy(qT2[:, (2 * t + 1) * 256:(2 * t + 2) * 256],
                                      qstage[64:128, :])

            # ---------------- fused P1/P2 pipeline ----------------
            kvp_store = {}

            def pad_eng(dg):
                # gpsimd cannot access PSUM; split PSUM reads DVE/ACT
                if dg % 2 == 0:
                    return nc.scalar
                return nc.vector

            def proj_op(dg, nh):
                if dg % 2 == 0:  # K tile -> st slots
                    kvp = ps.tile([128, 512], F32, tag="st", bufs=3,
                                  name="kvpk")
                else:
                    kvp = ps.tile([128, 512], F32, tag="kvp", bufs=1)
                first = True
                for s in range(2):
                    for wh, xh in ((0, 0), (0, 1), (1, 0)):
                        last = s == 1 and (wh, xh) == (1, 0)
                        _tag(nc.tensor.matmul(
                            kvp[:, :],
                            wkv[wh][:, 2 * s:2 * s + 2,
                                    dg * 128:(dg + 1) * 128],
                            xT[xh][:, 2 * s:2 * s + 2,
                                   nh * 512:(nh + 1) * 512],
                            start=first, stop=last,
                            perf_mode=mybir.MatmulPerfMode.DoubleRow),
                             f"proj({dg},{nh},{s})")
                        first = False
                kvp_store[(dg, nh)] = kvp

            def pad_op(dg, nh):
                pad_copy(dg, nh, kvp_store.pop((dg, nh)), pad_eng(dg))

            def p1_ops(t):
                dgk, dgv_ = 2 * t, 2 * t + 1
                mk = lambda f, *a: (lambda: f(*a))
                cstore = {}

                def conv_op(dg):
                    cstore[dg] = conv(dg)

                def ktail(tt):
                    k_tile(tt, *cstore.pop(2 * tt))

                def vtail(tt):
                    v_tile(tt, *cstore.pop(2 * tt + 1))

                ops = [
                    mk(proj_op, dgk, 0), mk(pad_op, dgk, 0),
                    mk(proj_op, dgv_, 0), mk(pad_op, dgv_, 0),
                    mk(proj_op, dgk, 1), mk(pad_op, dgk, 1),
                    mk(proj_op, dgv_, 1), mk(pad_op, dgv_, 1),
                    mk(conv_op, dgk), mk(ktail, t),
                    mk(conv_op, dgv_), mk(vtail, t),
                ]
                return ops

            ops0 = p1_ops(0)
            for op in ops0[0:8]:     # K0/V0 both halves
                op()
            diag_op(6, eng=nc.scalar)
            diag_op(7, eng=nc.scalar)
            ops0[8]()                # conv K0
            emit_qproj(0)
            emit_qproj(1)
            ops0[9]()                # ktail(0)
            ops0[10]()               # conv V0
            emit_qproj(2)
            emit_qproj(3)
            ops0[11]()               # vtail(0)
            diag_op(2, eng=nc.vector)
            diag_op(3, eng=nc.vector)

            # ---------------- normalize / attnT helpers ----------------
            def slot_of(chunk):
                h, lh = chunk // 2, chunk % 2
                return 4 * (h // 2) + 2 * lh + (h % 2)

            def norm_chunk(chunk):
                slot = slot_of(chunk)
                kk, sl = slot // 4, slot % 4
                uq, j = chunk // 4, chunk % 4
                src_v = avt[uq][:, j, 0:HD]
                src_z = avt[uq][:, j, HD:HD + 1]
                if chunk % 2 == 0:
                    _tag(nc.vector.reciprocal(zr[uq][:, j:j + 1], src_z),
                         f"recip({chunk})")
                    _tag(nc.vector.tensor_scalar_mul(
                        attn_sb[kk][:, sl, :], src_v,
                        zr[uq][:, j:j + 1]), f"nmul({chunk})")
                else:
                    _tag(nc.vector.reciprocal(zr[uq][:, j:j + 1], src_z),
                         f"recip({chunk})")
                    _tag(nc.scalar.mul(attn_sb[kk][:, sl, :], src_v,
                                  zr[uq][:, j:j + 1]), f"nmul({chunk})")

            vt2a_pre = {}

            def attnT_block(kk):
                # transposes + SBUF staging only (y matmuls emitted later)
                for lh in range(2):
                    if lh == 0:
                        vt2 = ps.tile([128, 128], F16, tag="kvp", bufs=1)
                    elif kk in vt2a_pre:
                        vt2 = vt2a_pre.pop(kk)
                    else:
                        vt2 = ps.tile([128, 128], F16, tag="av", bufs=4,
                                      name="vt2a")
                    asb = attn_sb[kk]
                    src_ap = bass.AP(
                        tensor=asb.tensor,
                        offset=asb.offset + 2 * lh * HD,
                        ap=[asb.ap[0], [1, 128]])
                    _tag(nc.tensor.transpose(vt2[:, :], src_ap, ident[:, :]),
                         f"atT({kk},{lh})")
                    if lh == 0:
                        _tag(nc.scalar.copy(
                            attnT[kk][:, 0:128], vt2[:, :]), f"atC({kk},{lh})")
                    else:
                        _tag(nc.vector.tensor_copy(
                            attnT[kk][:, 128:256], vt2[:, :]), f"atC({kk},{lh})")

            # Unit stream with av lagging sc by one; p1 micro-ops of tile
            # t+1 round-robined between tile t's units.  Groups t=0..2 run
            # mt-major; the last group runs uq-major so each uq's
            # normalize/attnT pipeline overlaps the next uq's score units.
            pending = []

            def drain_pending(keep):
                while len(pending) > keep:
                    av_unit(*pending.pop(0))

            for t in range(3):
                us = [(mt, uq) for mt in range(4 * t, 4 * t + 4)
                      for uq in range(4)]
                chunks = p1_ops(t + 1)
                nu, nch = len(us), len(chunks)
                ci = 0
                for i, u in enumerate(us):
                    mt, uq = u
                    ex = sc_unit(mt, uq)
                    pending.append((mt, uq, ex))
                    drain_pending(2)
                    want = ((i + 1) * nch) // nu
                    while ci < want:
                        chunks[ci]()
                        ci += 1

            # Last group: uq-major score stream with exps strictly
            # alternating engines; uq's normalize + attnT emitted AFTER
            # uq+1's score units so the exp cadence stays hot.
            for uq in range(4):
                for i, mt in enumerate(range(12, 16)):
                    force = "dve" if (uq * 4 + i) % 2 else "act"
                    ex = sc_unit(mt, uq, force_eng=force)
                    pending.append((mt, uq, ex))
                    drain_pending(2)
                if uq >= 1:
                    for j in range(4):
                        norm_chunk((uq - 1) * 4 + j)
                    attnT_block(uq - 1)
            drain_pending(0)
            # uq3 normalize must be emitted before its av slot is recycled
            for j in range(4):
                norm_chunk(12 + j)
            # vt2a(3) takes av slot 3 (WAR = uq3 norm reads, its true dep);
            # yp0-2 take the st slots freed by the last score units; yp3
            # takes av slot 0 (WAR = atC(0,1), long emitted).
            vt2a_pre[3] = ps.tile([128, 128], F16, tag="av", bufs=4,
                                  name="vt2a")
            yps = [ps.tile([128, 256], F32, tag="st", bufs=3,
                           name=f"yp{m}") for m in range(3)]
            yps.append(ps.tile([128, 256], F32, tag="av", bufs=4,
                               name="yp3"))

            def y_block(kk, with_copies=False):
                for m in range(4):
                    _tag(nc.tensor.matmul(
                        yps[m][:, :],
                        wo[:, kk, m * 128:(m + 1) * 128],
                        attnT[kk][:, :],
                        start=(kk == 0), stop=(kk == 3),
                        skip_group_check=True), f"y({kk},{m})")
                    if with_copies:
                        if m % 2 == 0:
                            _tag(nc.vector.tensor_scalar_add(
                                ysb[m // 2][:, m % 2, :], yps[m][:, :],
                                cst[:, 4 + m:5 + m]), f"ysb({m})")
                        else:
                            _tag(nc.scalar.add(
                                ysb[m // 2][:, m % 2, :], yps[m][:, :],
                                cst[:, 4 + m:5 + m]), f"ysb({m})")
                        if m == 1:
                            nc.sync.dma_start(
                                out=y_d[0:256, :].rearrange(
                                    "(m p) t -> p m t", p=128),
                                in_=ysb[0][:, :, :])

            for kk in range(3):
                y_block(kk)
            attnT_block(3)
            y_block(3, with_copies=True)
            if _dbg:
                nc.sync.dma_start(out=kT2_o[:, :], in_=kT2[:, :])
                nc.sync.dma_start(out=qT2_o[:, :], in_=qT2[:, :])
                nc.sync.dma_start(out=vaug_o[:, :],
                                  in_=vaug[:, :, :].rearrange("p a b -> p (a b)"))
                nc.sync.dma_start(out=asb_o[:, :],
                                  in_=attn_sb[:, :, :].rearrange("p a b -> p (a b)"))
            nc.gpsimd.dma_start(
                out=y_d[256:512, :].rearrange("(m p) t -> p m t", p=128),
                in_=ysb[1][:, :, :])

    nc.finalize()
    return nc


def _get_program():
    if "nc" not in _NC_CACHE:
        _NC_CACHE["nc"] = _build_program()
    return _NC_CACHE["nc"]


def _hilo8(a):
    """[rows, cols] f32 -> (hi, lo) fp8 e4m3 arrays."""
    import ml_dtypes
    f8 = ml_dtypes.float8_e4m3
    h = a.astype(f8)
    l = (a - h.astype(np.float32)).astype(f8)
    return np.ascontiguousarray(h), np.ascontiguousarray(l)


def _host_prep(x, wq, bq, wkv, bkv, dw_kernel, dw_bias, wo, bo):
    """Build the 8 per-core input maps."""
    x = np.ascontiguousarray(np.asarray(x, np.float32))
    wqh, wql = _hilo8(np.asarray(wq, np.float32) * 16.0)
    wo16 = np.asarray(wo, np.float32).astype(np.float16)
    bq = np.asarray(bq, np.float32)
    bkv = np.asarray(bkv, np.float32)
    dw_bias = np.asarray(dw_bias, np.float32)
    bo = np.asarray(bo, np.float32)
    dww = np.asarray(dw_kernel, np.float32).reshape(9, CH).T.copy()  # [1024, 9]

    # channel-tile processing order K0 V0 K1 V1 ... ; dg -> channel base
    dg_base = []
    for t in range(4):
        dg_base += [t * 128, DIM + t * 128]

    # wkv columns reordered to dg order
    wkv_f = np.asarray(wkv, np.float32)
    wkv_o = np.empty((DIM, CH), np.float32)
    for dg in range(8):
        b0 = dg_base[dg]
        wkv_o[:, dg * 128:(dg + 1) * 128] = wkv_f[:, b0:b0 + 128]
    wkvh, wkvl = _hilo8(wkv_o * 16.0)

    # bias plane: dw_bias + bkv * sum(valid taps), SAME padding aware,
    # rows in dg order
    oy = np.arange(KH)
    valid_y = (2 * oy[:, None] + np.arange(3)[None, :]) < H      # [16, 3]
    valid_x = valid_y.copy()
    wsum = np.zeros((CH, KH, KW), np.float32)
    for tap in range(9):
        dy, dx = tap // 3, tap % 3
        m2 = np.outer(valid_y[:, dy], valid_x[:, dx]).astype(np.float32)
        wsum += dww[:, tap][:, None, None] * m2[None, :, :]
    bpl_full = (dw_bias[:, None] + bkv[:, None] * wsum.reshape(CH, NS))
    bpl16 = np.empty((CH, NS), np.float16)
    for dg in range(8):
        b0 = dg_base[dg]
        bpl16[dg * 128:(dg + 1) * 128] = bpl_full[b0:b0 + 128]

    # conv diag values [128, 56] f32, dg-major then tap (taps 0-6 on PE)
    dgv = np.zeros((128, 56), np.float32)
    for dg in range(8):
        b0 = dg_base[dg]
        for tap in range(7):
            dgv[:, dg * 7 + tap] = dww[b0:b0 + 128, tap]

    cst = np.zeros((128, 40), np.float32)
    cst[:, 0:4] = 16.0 * bq.reshape(4, 128).T
    cst[:, 4:8] = bo.reshape(4, 128).T
    for dg in range(8):
        b0 = dg_base[dg]
        for tap in range(5, 9):
            cst[:, 8 + (tap - 5) * 8 + dg] = dww[b0:b0 + 128, tap]

    in_maps = []
    for c in range(8):
        b, j = c // 4, c % 4
        xh, xl = _hilo8(np.ascontiguousarray(x[b].reshape(L, DIM).T))
        in_maps.append({
            "xTh": xh, "xTl": xl,
            "xTch": np.ascontiguousarray(xh[:, j * 256:(j + 1) * 256]),
            "xTcl": np.ascontiguousarray(xl[:, j * 256:(j + 1) * 256]),
            "wkvh": wkvh, "wkvl": wkvl, "wqh": wqh, "wql": wql,
            "wo": wo16,
            "bpl": bpl16, "dgv": dgv, "cst": cst,
        })
    return in_maps


def kernel(**inputs) -> np.ndarray:
    nc = _get_program()
    in_maps = _host_prep(**inputs)
    res = run_bass_kernel_spmd(nc, in_maps, core_ids=list(range(8)))
    out = np.zeros((B, H, W, DIM), np.float32)
    flat = out.reshape(B, L, DIM)
    for c in range(8):
        b, j = c // 4, c % 4
        flat[b, j * 256:(j + 1) * 256, :] = res.results[c]["y"].T
    return out


# revision 30
# speedup vs baseline: 1.0777x; 1.0194x over previous
"""MobileMQA Trainium2 kernel v3 (8 NeuronCores, SPMD).

Reference computation (per batch b of 2):
  q  = x @ wq + bq                         [1024 tok, 512]
  kv = x @ wkv + bkv                       [1024 tok, 1024]
  kv = depthwise3x3_s2_same(kv) + dw_bias  [256 sp, 1024]
  k, v = split(kv)  -> shared-KV length M=2048 (channel fold)
  attn = softmax(q @ k^T * 0.125); out = attn @ v
  y = out @ wo + bo

Sharding: core c handles batch b=c//4, query chunk j=c%4 (256 tokens).
KV path (proj+conv) replicated across the 4 cores of a batch (MQA).

v3 design vs v2 (67765 ns):
  - Tail restructured: last mt-group runs uq-major; per-uq normalize +
    attnT transposes pipeline INTO the attention stream (prev-uq blocks
    emitted between next-uq score units). y-proj is a tight kk-major
    burst at the end; bo folds into the final PSUM->SBUF copies via cst
    (bias matmuls + ones16/bo16 inputs dropped); per-half output DMAs
    issue as soon as their ysb columns are written.
  - Conv diag weights built ON CHIP (DVE stride-0-broadcast
    tensor_tensor from a [128, 56] dgv input) instead of a 1.75MB dgw
    DMA; conv side-taps 7/8 moved DVE -> gpsimd; pad copies split
    ACT/DVE evenly.
  - kvsb conv-input buffers: only the SAME-pad border cells are
    memset (Pool) instead of the whole [128, 1089] plane; a separate
    [128, 512] zero strip feeds PE warm-up + PSUM zero-region matmuls.
  - Input DMAs coalesced/reordered by first-use (12 HWDGE dispatches).
"""
import sys

for _p in ("/opt/trn_rl_repo", "/opt/trn_rl_repo/concourse"):
    if _p not in sys.path:
        sys.path.insert(0, _p)

import numpy as np

import concourse.bass as bass
import concourse.mybir as mybir
import concourse.tile as tile
from concourse import bacc
from concourse.bass_utils import run_bass_kernel_spmd
from concourse.masks import make_identity

F32 = mybir.dt.float32
F16 = mybir.dt.float16
BF16 = mybir.dt.bfloat16
AF = mybir.ActivationFunctionType
ALU = mybir.AluOpType

DIM = 512
NH = 8
HD = 64
B, H, W = 2, 32, 32
L = H * W            # 1024 tokens per batch
KH = KW = 16
NS = KH * KW         # 256 conv-output spatial positions
M = NS * NH          # 2048 shared-KV positions
CH = 2 * DIM         # 1024 kv channels
SCALE = HD ** -0.5   # 0.125
PADW = 33            # padded conv input row (32 + 1 SAME pad after)
NPAD = PADW * PADW   # 1089

_NC_CACHE = {}
LABELS = {}


def _tag(bi, label):
    try:
        LABELS[bi.ins.name] = label
    except Exception:
        pass
    return bi


def _build_program():
    nc = bacc.Bacc(None)

    F8 = mybir.dt.float8e4
    xTh_d = nc.dram_tensor("xTh", [DIM, L], F8, kind="ExternalInput")
    xTl_d = nc.dram_tensor("xTl", [DIM, L], F8, kind="ExternalInput")
    xTch_d = nc.dram_tensor("xTch", [DIM, 256], F8, kind="ExternalInput")
    xTcl_d = nc.dram_tensor("xTcl", [DIM, 256], F8, kind="ExternalInput")
    wkvh_d = nc.dram_tensor("wkvh", [DIM, CH], F8, kind="ExternalInput")
    wkvl_d = nc.dram_tensor("wkvl", [DIM, CH], F8, kind="ExternalInput")
    wqh_d = nc.dram_tensor("wqh", [DIM, DIM], F8, kind="ExternalInput")
    wql_d = nc.dram_tensor("wql", [DIM, DIM], F8, kind="ExternalInput")
    wo_d = nc.dram_tensor("wo", [DIM, DIM], F16, kind="ExternalInput")
    bpl_d = nc.dram_tensor("bpl", [CH, NS], F16, kind="ExternalInput")
    dgv_d = nc.dram_tensor("dgv", [128, 56], F32, kind="ExternalInput")
    # cst cols: 0-3 bq tiles, 4-7 bo tiles, 8+ conv tap scalars
    cst_d = nc.dram_tensor("cst", [128, 40], F32, kind="ExternalInput")
    y_d = nc.dram_tensor("y", [DIM, 256], F32, kind="ExternalOutput")
    import os as _os
    _dbg = _os.environ.get("BASSDBG") == "1"
    if _dbg:
        kT2_o = nc.dram_tensor("kT2o", [64, M], F16, kind="ExternalOutput")
        qT2_o = nc.dram_tensor("qT2o", [64, M], F16, kind="ExternalOutput")
        vaug_o = nc.dram_tensor("vaugo", [128, 16 * (HD + 1)], BF16,
                                kind="ExternalOutput")
        asb_o = nc.dram_tensor("asbo", [128, 16 * HD], F16,
                               kind="ExternalOutput")

    with tile.TileContext(nc) as tc:
        with tc.tile_pool(name="wp", bufs=1) as wp, \
             tc.tile_pool(name="expp", bufs=8) as expp, \
             tc.tile_pool(name="tmpq", bufs=2) as tmppool, \
             tc.tile_pool(name="ps", bufs=1, space="PSUM") as ps:

            # ---------------- input DMAs (priority order) ----------------
            cst = wp.tile([128, 40], F32, tag="cst")
            rr = lambda d: d[:, :].rearrange("(k p) t -> p k t", p=128)
            xTc = [wp.tile([128, 4, 256], F8, tag=f"xTc{h}", name=f"xTc{h}")
                   for h in range(2)]
            wq = [wp.tile([128, 4, DIM], F8, tag=f"wq{h}", name=f"wq{h}")
                  for h in range(2)]
            xT = [wp.tile([128, 4, L], F8, tag=f"xT{h}", name=f"xT{h}")
                  for h in range(2)]
            wkv = [wp.tile([128, 4, CH], F8, tag=f"wkv{h}", name=f"wkv{h}")
                   for h in range(2)]
            dgv = wp.tile([128, 56], F32, tag="dgv")
            bpl = wp.tile([128, 8, NS], F16, tag="bpl")
            wo = wp.tile([128, 4, DIM], F16, tag="wo")
            xT_r = [rr(xTh_d), rr(xTl_d)]
            wkv_r = [rr(wkvh_d), rr(wkvl_d)]
            xTc_r = [rr(xTch_d), rr(xTcl_d)]
            wq_r = [rr(wqh_d), rr(wql_d)]
            bpl_r = bpl_d[:, :].rearrange("(t p) s -> p t s", p=128)

            # critical-path order: K0/V0 proj operands, q operands, then
            # second-half tokens, conv bias plane, remaining kv weights,
            # y-proj weights last.
            for h in range(2):
                nc.sync.dma_start(out=wkv[h][:, :, 0:256],
                                  in_=wkv_r[h][:, :, 0:256])
            nc.sync.dma_start(out=dgv, in_=dgv_d[:, :])
            for h in range(2):
                nc.sync.dma_start(out=xT[h][:, :, 0:512],
                                  in_=xT_r[h][:, :, 0:512])
            nc.sync.dma_start(out=bpl[:, 0:2, :], in_=bpl_r[:, 0:2, :])
            for h in range(2):
                nc.sync.dma_start(out=xT[h][:, :, 512:L],
                                  in_=xT_r[h][:, :, 512:L])
            for h in range(2):
                nc.sync.dma_start(out=xTc[h], in_=xTc_r[h])
            for h in range(2):
                nc.sync.dma_start(out=wq[h], in_=wq_r[h])
            nc.sync.dma_start(out=cst, in_=cst_d[:, :])
            nc.sync.dma_start(out=bpl[:, 2:8, :], in_=bpl_r[:, 2:8, :])
            for h in range(2):
                nc.sync.dma_start(out=wkv[h][:, :, 256:CH],
                                  in_=wkv_r[h][:, :, 256:CH])
            nc.sync.dma_start(out=wo,
                              in_=wo_d[:, :].rearrange("(k p) c -> p k c", p=128))

            # ---------------- persistent SBUF state ----------------
            ident = wp.tile([128, 128], F16, tag="ident")
            make_identity(nc, ident)
            # preload exp ACT table during the DMA window
            warm = wp.tile([1, 1], F32, tag="warm")
            nc.vector.memset(warm, 0.0)
            nc.scalar.activation(warm[:, :], warm[:, :], AF.Exp)

            kT2 = wp.tile([64, M], F16, tag="kT2")
            qT2 = wp.tile([64, M], F16, tag="qT2")
            vaug = wp.tile([128, 16, HD + 1], BF16, tag="vaug")
            nc.vector.memset(vaug[:, :, HD:HD + 1], 1.0)
            attnT = [wp.tile([128, 256], F16, tag=f"attnT{k}",
                              name=f"attnT{k}") for k in range(4)]
            attn_sb = [wp.tile([128, 4, HD], F16, tag=f"attn_sb{k}",
                                name=f"attn_sb{k}") for k in range(4)]
            zr = [wp.tile([128, 4], F32, tag=f"zr{k}", name=f"zr{k}")
                  for k in range(4)]
            vacc = wp.tile([128, NS], F16, tag="vacc")
            ysb = [wp.tile([128, 2, 256], F32, tag=f"ysb{h}",
                          name=f"ysb{h}") for h in range(2)]

            # zero strip for PE warm-up and PSUM zero-region matmuls
            zstrip = wp.tile([128, 512], F16, tag="zstrip")
            nc.gpsimd.memset(zstrip[:, :], 0.0)

            # conv-input buffers: only SAME-pad border cells are zeroed
            # (col 32 of rows 0-32, then row 32 cols 0-31); the interior
            # is fully overwritten by pad_copy each use.
            kvsb = []
            for i in range(2):
                kb = wp.tile([128, NPAD], F16, tag=f"kvsb{i}",
                             name=f"kvsb{i}")
                colpad = bass.AP(tensor=kb.tensor, offset=kb.offset + 32,
                                 ap=[kb.ap[0], [PADW, PADW]])
                rowpad = bass.AP(tensor=kb.tensor,
                                 offset=kb.offset + PADW * 32,
                                 ap=[kb.ap[0], [1, 32]])
                nc.gpsimd.memset(colpad, 0.0)
                nc.gpsimd.memset(rowpad, 0.0)
                kvsb.append(kb)

            # conv diag weights built on-chip: dgt[:, dg*7+tap, :] =
            # diag(dgv[:, dg*7+tap]) via stride-0-broadcast tensor_tensor
            dgt = wp.tile([128, 56, 128], F16, tag="dgt")

            def diag_op(dg, eng=None):
                if eng is nc.scalar:
                    for tap in range(7):
                        i = dg * 7 + tap
                        _tag(nc.scalar.mul(dgt[:, i, :], ident[:, :],
                                           dgv[:, i:i + 1]), f"diag({dg})")
                    return
                id_b = bass.AP(tensor=ident.tensor, offset=ident.offset,
                               ap=[ident.ap[0], [0, 7], [1, 128]])
                dg_b = bass.AP(tensor=dgv.tensor, offset=dgv.offset + 7 * dg,
                               ap=[dgv.ap[0], [1, 7], [0, 128]])
                _tag((eng or nc.gpsimd).tensor_tensor(
                    dgt[:, dg * 7:(dg + 1) * 7, :],
                    id_b, dg_b, op=ALU.mult), f"diag({dg})")

            diag_op(0, eng=nc.vector)
            diag_op(1, eng=nc.vector)
            diag_op(4)
            diag_op(5)
            diag_late = {}

            # PE warm-up: keep the array busy through the DMA window so the
            # p-state ramp completes before the real matmuls arrive.
            for _w in range(4):
                wmm = ps.tile([128, 512], F32, tag="kvp", bufs=1)
                nc.tensor.matmul(wmm[:, :], ident[:, :], zstrip[:, :],
                                 start=True, stop=True)

            # ---------------- helpers ----------------
            def pad_copy(dg, nh, kvp, eng):
                kb = kvsb[dg % 2]
                dst = bass.AP(tensor=kb.tensor,
                              offset=kb.offset + PADW * 16 * nh,
                              ap=[kb.ap[0], [PADW, 16], [1, 32]])
                src = kvp[:, :].rearrange("p (a b) -> p a b", b=32)
                if eng is nc.scalar:
                    _tag(eng.mul(dst, src, 1.0 / 16.0), f"pad({dg},{nh})")
                else:
                    _tag(eng.tensor_scalar_mul(dst, src, 1.0 / 16.0),
                         f"pad({dg},{nh})")

            tmpp = tmppool

            def conv(dg):
                """Conv taps 0-6 on PE (PSUM cvp); taps 7,8 + bias into an
                SBUF side-accumulator on gpsimd, merged at the K/V stt."""
                kb = kvsb[dg % 2]
                tmp = tmpp.tile([128, NS], F32, tag="tmp")
                for tap in (7, 8):
                    dy, dx = tap // 3, tap % 3
                    win = bass.AP(tensor=kb.tensor,
                                  offset=kb.offset + PADW * dy + dx,
                                  ap=[kb.ap[0], [2 * PADW, KH], [2, KW]])
                    col = 8 + (tap - 5) * 8 + dg
                    in1 = bpl[:, dg, :] if tap == 7 else tmp[:, :]
                    _tag(nc.vector.scalar_tensor_tensor(
                        tmp[:, :], win, cst[:, col:col + 1], in1,
                        op0=ALU.mult, op1=ALU.add), f"ctap({dg},{tap})")
                cvp = ps.tile([128, NS], F32, tag="kvp", bufs=1)
                for tap in range(7):
                    dy, dx = tap // 3, tap % 3
                    win = bass.AP(tensor=kb.tensor,
                                  offset=kb.offset + PADW * dy + dx,
                                  ap=[kb.ap[0], [2 * PADW, KH], [2, KW]])
                    _tag(nc.tensor.matmul(cvp[:, :], dgt[:, dg * 7 + tap, :], win,
                                     start=(tap == 0), stop=(tap == 6)),
                         f"cmm({dg},{tap})")
                return cvp, tmp

            def k_tile(t, cvp, tmp):
                # gpsimd cannot read PSUM: merge to SBUF, then cross-copy
                kacc = tmpp.tile([128, NS], F16, tag="kacc")
                _tag(nc.vector.scalar_tensor_tensor(
                    kacc[:, :], cvp[:, :], 1.0, tmp[:, :],
                    op0=ALU.mult, op1=ALU.add), f"kmerge({t})")
                _tag(nc.gpsimd.tensor_copy(kT2[:, (2 * t) * 256:(2 * t + 1) * 256],
                                      kacc[0:64, :]), f"kcopy({t},0)")
                _tag(nc.gpsimd.tensor_copy(
                    kT2[:, (2 * t + 1) * 256:(2 * t + 2) * 256],
                    kacc[64:128, :]), f"kcopy({t},1)")

            def v_tile(t, cvp, tmp):
                _tag(nc.vector.scalar_tensor_tensor(
                    vacc[:, :], cvp[:, :], 1.0, tmp[:, :],
                    op0=ALU.mult, op1=ALU.add), f"vmerge({t})")
                for gi in range(2):
                    vt = ps.tile([128, 128], F16, tag="kvp", bufs=1)
                    for sh in range(2):
                        nc.tensor.transpose(
                            vt[:, sh * 64:(sh + 1) * 64],
                            vacc[gi * 64:(gi + 1) * 64,
                                 sh * 128:(sh + 1) * 128],
                            ident[gi * 64:(gi + 1) * 64,
                                  gi * 64:(gi + 1) * 64])
                    mt0 = 4 * t + 2 * gi
                    _tag(nc.vector.tensor_copy(
                        vaug[:, mt0:mt0 + 2, 0:HD],
                        vt[:, :].rearrange("p (a b) -> p a b", b=64)),
                         f"vcopy({t},{gi})")

            # per-uq AV accumulators, one PSUM bank each so normalize
            # reads of uq never serialize against av writes of uq+1
            avt = [ps.tile([128, 4, 128], F32, tag="av", bufs=4,
                           name=f"av{u}") for u in range(4)]
            for u in range(4):
                zmm = bass.AP(tensor=avt[u].tensor, offset=avt[u].offset,
                              ap=[avt[u].ap[0], [1, 512]])
                nc.tensor.matmul(zmm, ident[:, :], zstrip[:, :],
                                 start=True, stop=True, skip_group_check=True)

            ucount = [0]

            def sc_unit(mt, uq, force_eng=None):
                st = ps.tile([128, 512], F32, tag="st", bufs=3)
                _tag(nc.tensor.matmul(st[:, :],
                                 kT2[:, mt * 128:(mt + 1) * 128],
                                 qT2[:, uq * 512:(uq + 1) * 512],
                                 start=True, stop=True), f"sc({mt},{uq})")
                ucount[0] += 1
                if force_eng == "split":
                    ex = expp.tile([128, 512], BF16, tag="ex")
                    _tag(nc.scalar.activation(ex[:, 0:256], st[:, 0:256],
                                              AF.Exp, scale=float(SCALE)),
                         f"expA({mt},{uq})")
                    exd = ex[:, 256:512].bitcast(mybir.dt.int16)
                    _tag(nc.vector.tensor_scalar(
                        exd, st[:, 256:512], 23.08294, 16248.78,
                        op0=ALU.mult, op1=ALU.add), f"expD({mt},{uq})")
                    return ex
                dve = ucount[0] % 3 == 0
                if force_eng is not None:
                    dve = force_eng == "dve"
                if dve:
                    # Schraudolph exp on DVE: bf16 bits = s*scale*184.66 +
                    # (16256 - 7.22); int16 convert, bitcast view as bf16.
                    exd = expp.tile([128, 512], mybir.dt.int16, tag="ex",
                                    name="exd")
                    _tag(nc.vector.tensor_scalar(
                        exd[:, :], st[:, :], 23.08294, 16248.78,
                        op0=ALU.mult, op1=ALU.add), f"expD({mt},{uq})")
                    return exd.bitcast(BF16)
                ex = expp.tile([128, 512], BF16, tag="ex")
                _tag(nc.scalar.activation(ex[:, :], st[:, :], AF.Exp,
                                     scale=float(SCALE)), f"expA({mt},{uq})")
                return ex

            def av_unit(mt, uq, ex):
                for j in range(4):
                    _tag(nc.tensor.matmul(avt[uq][:, j, 0:HD + 1],
                                     ex[:, j * 128:(j + 1) * 128],
                                     vaug[:, mt, :],
                                     start=False, stop=(mt == 15),
                                     skip_group_check=True), f"av({mt},{uq},{j})")

            # ---------------- q projection ----------------
            def emit_qproj(t):
                eng = nc.scalar if t % 2 == 0 else nc.vector
                qp = ps.tile([128, 512], F32, tag="st", bufs=3)
                qpv = qp[:, 0:256]
                first = True
                for s in range(2):
                    for wh, xh in ((0, 0), (0, 1), (1, 0)):
                        last = s == 1 and (wh, xh) == (1, 0)
                        _tag(nc.tensor.matmul(
                            qpv,
                            wq[wh][:, 2 * s:2 * s + 2,
                                   t * 128:(t + 1) * 128],
                            xTc[xh][:, 2 * s:2 * s + 2, :],
                            start=first, stop=last,
                            perf_mode=mybir.MatmulPerfMode.DoubleRow),
                             f"qp({t},{s})")
                        first = False
                if eng is nc.scalar:
                    nc.scalar.add(qpv, qpv, cst[:, t:t + 1])
                    nc.scalar.mul(qT2[:, (2 * t) * 256:(2 * t + 1) * 256],
                                  qp[0:64, 0:256], 1.0 / 16.0)
                else:
                    nc.vector.tensor_scalar_add(qpv, qpv, cst[:, t:t + 1])
                    nc.vector.tensor_scalar_mul(
                        qT2[:, (2 * t) * 256:(2 * t + 1) * 256],
                        qp[0:64, 0:256], 1.0 / 16.0)
                qstage = tmpp.tile([128, 256], F16, tag="kacc", name="qstage")
                if eng is nc.scalar:
                    nc.scalar.mul(qstage[64:128, :],
                                  qp[64:128, 0:256], 1.0 / 16.0)
                else:
                    nc.vector.tensor_scalar_mul(qstage[64:128, :],
                                                qp[64:128, 0:256], 1.0 / 16.0)
                nc.gpsimd.tensor_copy(qT2[:, (2 * t + 1) * 256:(2 * t + 2) * 256],
                                      qstage[64:128, :])

            # ---------------- fused P1/P2 pipeline ----------------
            kvp_store = {}

            def pad_eng(dg):
                # gpsimd cannot access PSUM; split PSUM reads DVE/ACT
                if dg % 2 == 0:
                    return nc.scalar
                return nc.vector

            def proj_op(dg, nh):
                if dg % 2 == 0:  # K tile -> st slots
                    kvp = ps.tile([128, 512], F32, tag="st", bufs=3,
                                  name="kvpk")
                else:
                    kvp = ps.tile([128, 512], F32, tag="kvp", bufs=1)
                first = True
                for s in range(2):
                    for wh, xh in ((0, 0), (0, 1), (1, 0)):
                        last = s == 1 and (wh, xh) == (1, 0)
                        _tag(nc.tensor.matmul(
                            kvp[:, :],
                            wkv[wh][:, 2 * s:2 * s + 2,
                                    dg * 128:(dg + 1) * 128],
                            xT[xh][:, 2 * s:2 * s + 2,
                                   nh * 512:(nh + 1) * 512],
                            start=first, stop=last,
                            perf_mode=mybir.MatmulPerfMode.DoubleRow),
                             f"proj({dg},{nh},{s})")
                        first = False
                kvp_store[(dg, nh)] = kvp

            def pad_op(dg, nh):
                pad_copy(dg, nh, kvp_store.pop((dg, nh)), pad_eng(dg))

            def p1_ops(t):
                dgk, dgv_ = 2 * t, 2 * t + 1
                mk = lambda f, *a: (lambda: f(*a))
                cstore = {}

                def conv_op(dg):
                    cstore[dg] = conv(dg)

                def ktail(tt):
                    k_tile(tt, *cstore.pop(2 * tt))

                def vtail(tt):
                    v_tile(tt, *cstore.pop(2 * tt + 1))

                ops = [
                    mk(proj_op, dgk, 0), mk(pad_op, dgk, 0),
                    mk(proj_op, dgv_, 0), mk(pad_op, dgv_, 0),
                    mk(proj_op, dgk, 1), mk(pad_op, dgk, 1),
                    mk(proj_op, dgv_, 1), mk(pad_op, dgv_, 1),
                    mk(conv_op, dgk), mk(ktail, t),
                    mk(conv_op, dgv_), mk(vtail, t),
                ]
                for dgx in diag_late.pop(t, ()):
                    ops.append(mk(diag_op, dgx))
                return ops

            ops0 = p1_ops(0)
            for op in ops0[0:8]:     # K0/V0 both halves
                op()
            diag_op(6, eng=nc.scalar)
            diag_op(7, eng=nc.scalar)
            ops0[8]()                # conv K0
            ops0[9]()                # ktail(0)
            emit_qproj(0)
            emit_qproj(1)
            ops0[10]()               # conv V0
            emit_qproj(2)
            emit_qproj(3)
            ops0[11]()               # vtail(0)
            diag_op(2, eng=nc.vector)
            diag_op(3, eng=nc.vector)

            # ---------------- normalize / attnT helpers ----------------
            def slot_of(chunk):
                h, lh = chunk // 2, chunk % 2
                return 4 * (h // 2) + 2 * lh + (h % 2)

            def norm_chunk(chunk):
                slot = slot_of(chunk)
                kk, sl = slot // 4, slot % 4
                uq, j = chunk // 4, chunk % 4
                src_v = avt[uq][:, j, 0:HD]
                src_z = avt[uq][:, j, HD:HD + 1]
                if chunk % 2 == 0:
                    _tag(nc.vector.reciprocal(zr[uq][:, j:j + 1], src_z),
                         f"recip({chunk})")
                    _tag(nc.vector.tensor_scalar_mul(
                        attn_sb[kk][:, sl, :], src_v,
                        zr[uq][:, j:j + 1]), f"nmul({chunk})")
                else:
                    _tag(nc.vector.reciprocal(zr[uq][:, j:j + 1], src_z),
                         f"recip({chunk})")
                    _tag(nc.scalar.mul(attn_sb[kk][:, sl, :], src_v,
                                  zr[uq][:, j:j + 1]), f"nmul({chunk})")

            vt2a_pre = {}

            def attnT_block(kk):
                # transposes + SBUF staging only (y matmuls emitted later)
                for lh in range(2):
                    if lh == 0:
                        vt2 = ps.tile([128, 128], F16, tag="kvp", bufs=1)
                    elif kk in vt2a_pre:
                        vt2 = vt2a_pre.pop(kk)
                    else:
                        vt2 = ps.tile([128, 128], F16, tag="av", bufs=4,
                                      name="vt2a")
                    asb = attn_sb[kk]
                    src_ap = bass.AP(
                        tensor=asb.tensor,
                        offset=asb.offset + 2 * lh * HD,
                        ap=[asb.ap[0], [1, 128]])
                    _tag(nc.tensor.transpose(vt2[:, :], src_ap, ident[:, :]),
                         f"atT({kk},{lh})")
                    if lh == 0:
                        _tag(nc.scalar.copy(
                            attnT[kk][:, 0:128], vt2[:, :]), f"atC({kk},{lh})")
                    else:
                        _tag(nc.vector.tensor_copy(
                            attnT[kk][:, 128:256], vt2[:, :]), f"atC({kk},{lh})")

            # Unit stream with av lagging sc by one; p1 micro-ops of tile
            # t+1 round-robined between tile t's units.  Groups t=0..2 run
            # mt-major; the last group runs uq-major so each uq's
            # normalize/attnT pipeline overlaps the next uq's score units.
            pending = []

            def drain_pending(keep):
                while len(pending) > keep:
                    av_unit(*pending.pop(0))

            for t in range(3):
                us = [(mt, uq) for mt in range(4 * t, 4 * t + 4)
                      for uq in range(4)]
                chunks = p1_ops(t + 1)
                nu, nch = len(us), len(chunks)
                ci = 0
                for i, u in enumerate(us):
                    mt, uq = u
                    ex = sc_unit(mt, uq)
                    pending.append((mt, uq, ex))
                    drain_pending(2)
                    want = ((i + 1) * nch) // nu
                    while ci < want:
                        chunks[ci]()
                        ci += 1

            # Last group: uq-major score stream with exps strictly
            # alternating engines; uq's normalize + attnT emitted AFTER
            # uq+1's score units so the exp cadence stays hot.
            for uq in range(4):
                for i, mt in enumerate(range(12, 16)):
                    force = "dve" if (uq * 4 + i) % 2 else "act"
                    ex = sc_unit(mt, uq, force_eng=force)
                    pending.append((mt, uq, ex))
                    drain_pending(2)
                if uq >= 1:
                    for j in range(4):
                        norm_chunk((uq - 1) * 4 + j)
                    attnT_block(uq - 1)
            drain_pending(0)
            # uq3 normalize must be emitted before its av slot is recycled
            for j in range(4):
                norm_chunk(12 + j)
            # vt2a(3) takes av slot 3 (WAR = uq3 norm reads, its true dep);
            # yp0-2 take the st slots freed by the last score units; yp3
            # takes av slot 0 (WAR = atC(0,1), long emitted).
            vt2a_pre[3] = ps.tile([128, 128], F16, tag="av", bufs=4,
                                  name="vt2a")
            yps = [ps.tile([128, 256], F32, tag="st", bufs=3,
                           name=f"yp{m}") for m in range(3)]
            yps.append(ps.tile([128, 256], F32, tag="av", bufs=4,
                               name="yp3"))

            def y_block(kk, with_copies=False):
                for m in range(4):
                    _tag(nc.tensor.matmul(
                        yps[m][:, :],
                        wo[:, kk, m * 128:(m + 1) * 128],
                        attnT[kk][:, :],
                        start=(kk == 0), stop=(kk == 3),
                        skip_group_check=True), f"y({kk},{m})")
                    if with_copies:
                        if m % 2 == 0:
                            _tag(nc.vector.tensor_scalar_add(
                                ysb[m // 2][:, m % 2, :], yps[m][:, :],
                                cst[:, 4 + m:5 + m]), f"ysb({m})")
                        else:
                            _tag(nc.scalar.add(
                                ysb[m // 2][:, m % 2, :], yps[m][:, :],
                                cst[:, 4 + m:5 + m]), f"ysb({m})")
                        if m == 1:
                            nc.sync.dma_start(
                                out=y_d[0:256, :].rearrange(
                                    "(m p) t -> p m t", p=128),
                                in_=ysb[0][:, :, :])

            for kk in range(3):
                y_block(kk)
            attnT_block(3)
            y_block(3, with_copies=True)
            if _dbg:
                nc.sync.dma_start(out=kT2_o[:, :], in_=kT2[:, :])
                nc.sync.dma_start(out=qT2_o[:, :], in_=qT2[:, :])
                nc.sync.dma_start(out=vaug_o[:, :],
                                  in_=vaug[:, :, :].rearrange("p a b -> p (a b)"))
                nc.sync.dma_start(out=asb_o[:, :],
                                  in_=attn_sb[:, :, :].rearrange("p a b -> p (a b)"))
            nc.gpsimd.dma_start(
                out=y_d[256:512, :].rearrange("(m p) t -> p m t", p=128),
                in_=ysb[1][:, :, :])

    nc.finalize()
    return nc


def _get_program():
    if "nc" not in _NC_CACHE:
        _NC_CACHE["nc"] = _build_program()
    return _NC_CACHE["nc"]


def _hilo8(a):
    """[rows, cols] f32 -> (hi, lo) fp8 e4m3 arrays."""
    import ml_dtypes
    f8 = ml_dtypes.float8_e4m3
    h = a.astype(f8)
    l = (a - h.astype(np.float32)).astype(f8)
    return np.ascontiguousarray(h), np.ascontiguousarray(l)


def _host_prep(x, wq, bq, wkv, bkv, dw_kernel, dw_bias, wo, bo):
    """Build the 8 per-core input maps."""
    x = np.ascontiguousarray(np.asarray(x, np.float32))
    wqh, wql = _hilo8(np.asarray(wq, np.float32) * 16.0)
    wo16 = np.asarray(wo, np.float32).astype(np.float16)
    bq = np.asarray(bq, np.float32)
    bkv = np.asarray(bkv, np.float32)
    dw_bias = np.asarray(dw_bias, np.float32)
    bo = np.asarray(bo, np.float32)
    dww = np.asarray(dw_kernel, np.float32).reshape(9, CH).T.copy()  # [1024, 9]

    # channel-tile processing order K0 V0 K1 V1 ... ; dg -> channel base
    dg_base = []
    for t in range(4):
        dg_base += [t * 128, DIM + t * 128]

    # wkv columns reordered to dg order
    wkv_f = np.asarray(wkv, np.float32)
    wkv_o = np.empty((DIM, CH), np.float32)
    for dg in range(8):
        b0 = dg_base[dg]
        wkv_o[:, dg * 128:(dg + 1) * 128] = wkv_f[:, b0:b0 + 128]
    wkvh, wkvl = _hilo8(wkv_o * 16.0)

    # bias plane: dw_bias + bkv * sum(valid taps), SAME padding aware,
    # rows in dg order
    oy = np.arange(KH)
    valid_y = (2 * oy[:, None] + np.arange(3)[None, :]) < H      # [16, 3]
    valid_x = valid_y.copy()
    wsum = np.zeros((CH, KH, KW), np.float32)
    for tap in range(9):
        dy, dx = tap // 3, tap % 3
        m2 = np.outer(valid_y[:, dy], valid_x[:, dx]).astype(np.float32)
        wsum += dww[:, tap][:, None, None] * m2[None, :, :]
    bpl_full = (dw_bias[:, None] + bkv[:, None] * wsum.reshape(CH, NS))
    bpl16 = np.empty((CH, NS), np.float16)
    for dg in range(8):
        b0 = dg_base[dg]
        bpl16[dg * 128:(dg + 1) * 128] = bpl_full[b0:b0 + 128]

    # conv diag values [128, 56] f32, dg-major then tap (taps 0-6 on PE)
    dgv = np.zeros((128, 56), np.float32)
    for dg in range(8):
        b0 = dg_base[dg]
        for tap in range(7):
            dgv[:, dg * 7 + tap] = dww[b0:b0 + 128, tap]

    cst = np.zeros((128, 40), np.float32)
    cst[:, 0:4] = 16.0 * bq.reshape(4, 128).T
    cst[:, 4:8] = bo.reshape(4, 128).T
    for dg in range(8):
        b0 = dg_base[dg]
        for tap in range(5, 9):
            cst[:, 8 + (tap - 5) * 8 + dg] = dww[b0:b0 + 128, tap]

    in_maps = []
    for c in range(8):
        b, j = c // 4, c % 4
        xh, xl = _hilo8(np.ascontiguousarray(x[b].reshape(L, DIM).T))
        in_maps.append({
            "xTh": xh, "xTl": xl,
            "xTch": np.ascontiguousarray(xh[:, j * 256:(j + 1) * 256]),
            "xTcl": np.ascontiguousarray(xl[:, j * 256:(j + 1) * 256]),
            "wkvh": wkvh, "wkvl": wkvl, "wqh": wqh, "wql": wql,
            "wo": wo16,
            "bpl": bpl16, "dgv": dgv, "cst": cst,
        })
    return in_maps


def kernel(**inputs) -> np.ndarray:
    nc = _get_program()
    in_maps = _host_prep(**inputs)
    res = run_bass_kernel_spmd(nc, in_maps, core_ids=list(range(8)))
    out = np.zeros((B, H, W, DIM), np.float32)
    flat = out.reshape(B, L, DIM)
    for c in range(8):
        b, j = c // 4, c % 4
        flat[b, j * 256:(j + 1) * 256, :] = res.results[c]["y"].T
    return out


# revision 39
# speedup vs baseline: 1.0807x; 1.0028x over previous
"""MobileMQA Trainium2 kernel v4 (8 NeuronCores, SPMD).

Reference computation (per batch b of 2):
  q  = x @ wq + bq                         [1024 tok, 512]
  kv = x @ wkv + bkv                       [1024 tok, 1024]
  kv = depthwise3x3_s2_same(kv) + dw_bias  [256 sp, 1024]
  k, v = split(kv)  -> shared-KV length M=2048 (channel fold)
  attn = softmax(q @ k^T * 0.125); out = attn @ v
  y = out @ wo + bo

Sharding: core c handles batch b=c//4, query chunk j=c%4 (256 tokens).
KV path (proj+conv) replicated across the 4 cores of a batch (MQA).

v4 design vs v2 (67765 -> 62706 ns in TimelineSim):
  - KV and Q projections in fp8 e4m3 DoubleRow (0.5 cycles/row,
    contracting 256/pass): x and the weights ship as hi/lo residual
    pairs, computed as Whi*xhi + Whi*xlo + Wlo*xhi (3 DoubleRow matmuls
    per 256-contraction vs 4 fp16 ones).  Weights are pre-scaled x16 on
    the host so the lo residuals clear e4m3's subnormal floor (without
    this the he-init weights quantize at ~0.9% and softmax tail
    amplification pushes end-to-end error over the 2e-2 gate); the 1/16
    compensates for free inside the existing PSUM->SBUF pad/q copies.
    Scores/AV/conv/y-proj stay fp16/bf16 (fp8 there fails the error
    budget: softmax amplifies logit-scale errors ~3x).
  - Tail restructured: last mt-group runs uq-major; per-uq normalize +
    attnT transposes are emitted after the NEXT uq's score units so the
    exp engines stay saturated; y-proj runs as a kk-major burst with
    per-half output DMAs.  bo folds into the final copies via cst.
  - AV accumulators split into four single-bank PSUM tiles (one per uq)
    so normalize reads never serialize against the next uq's AV writes
    (PSUM dependency tracking is coarse); attn_sb/attnT/zr/ysb split
    per-block for the same reason; score pool st shrinks 4->3 bufs to
    fund the fourth av bank.  attnT lh=1 transposes and the y-proj
    accumulators reuse av-pool slots whose WARs coincide with their true
    dependencies (allocation order matters: a slot's next tile must be
    allocated only after the previous tile's readers are emitted, else
    the pool WAR chain can deadlock).
  - Conv diag weights built on-chip from a [128, 56] dgv input
    (stride-0-broadcast tensor_tensor: dg 0-1 on DVE, 4-7 on Pool's
    idle window) instead of a 1.75MB dgw DMA.  kvsb conv-input buffers
    memset only in the SAME-pad border cells; a [128, 512] zero strip
    feeds warm-up and PSUM zero-region matmuls.
  - Input DMAs coalesced and ordered by first-use (cst/dgv tiny ones
    early - cst gates the conv side-taps; bpl[0:2] before the xT second
    halves); q-proj elementwise moved to ACT's idle head window.
  - Known ISA limits hit: Pool has no TensorScalarPtr and no fp32-scale
    Activation; DVE TensorScalar has no divide; DMA APs max 3 free
    dims; PSUM pool tiles are bank-aligned (8 x 2KB total).
"""
import sys

for _p in ("/opt/trn_rl_repo", "/opt/trn_rl_repo/concourse"):
    if _p not in sys.path:
        sys.path.insert(0, _p)

import numpy as np

import concourse.bass as bass
import concourse.mybir as mybir
import concourse.tile as tile
from concourse import bacc
from concourse.bass_utils import run_bass_kernel_spmd
from concourse.masks import make_identity

F32 = mybir.dt.float32
F16 = mybir.dt.float16
BF16 = mybir.dt.bfloat16
AF = mybir.ActivationFunctionType
ALU = mybir.AluOpType

DIM = 512
NH = 8
HD = 64
B, H, W = 2, 32, 32
L = H * W            # 1024 tokens per batch
KH = KW = 16
NS = KH * KW         # 256 conv-output spatial positions
M = NS * NH          # 2048 shared-KV positions
CH = 2 * DIM         # 1024 kv channels
SCALE = HD ** -0.5   # 0.125
PADW = 33            # padded conv input row (32 + 1 SAME pad after)
NPAD = PADW * PADW   # 1089

_NC_CACHE = {}
LABELS = {}


def _tag(bi, label):
    try:
        LABELS[bi.ins.name] = label
    except Exception:
        pass
    return bi


def _build_program():
    nc = bacc.Bacc(None)

    F8 = mybir.dt.float8e4
    xTh_d = nc.dram_tensor("xTh", [DIM, L], F8, kind="ExternalInput")
    xTl_d = nc.dram_tensor("xTl", [DIM, L], F8, kind="ExternalInput")
    xTch_d = nc.dram_tensor("xTch", [DIM, 256], F8, kind="ExternalInput")
    xTcl_d = nc.dram_tensor("xTcl", [DIM, 256], F8, kind="ExternalInput")
    wkvh_d = nc.dram_tensor("wkvh", [DIM, CH], F8, kind="ExternalInput")
    wkvl_d = nc.dram_tensor("wkvl", [DIM, CH], F8, kind="ExternalInput")
    wqh_d = nc.dram_tensor("wqh", [DIM, DIM], F8, kind="ExternalInput")
    wql_d = nc.dram_tensor("wql", [DIM, DIM], F8, kind="ExternalInput")
    wo_d = nc.dram_tensor("wo", [DIM, DIM], F16, kind="ExternalInput")
    bpl_d = nc.dram_tensor("bpl", [CH, NS], F16, kind="ExternalInput")
    dgv_d = nc.dram_tensor("dgv", [128, 56], F32, kind="ExternalInput")
    # cst cols: 0-3 bq tiles, 4-7 bo tiles, 8+ conv tap scalars
    cst_d = nc.dram_tensor("cst", [128, 40], F32, kind="ExternalInput")
    y_d = nc.dram_tensor("y", [DIM, 256], F32, kind="ExternalOutput")
    import os as _os
    _dbg = _os.environ.get("BASSDBG") == "1"
    if _dbg:
        kT2_o = nc.dram_tensor("kT2o", [64, M], F16, kind="ExternalOutput")
        qT2_o = nc.dram_tensor("qT2o", [64, M], F16, kind="ExternalOutput")
        vaug_o = nc.dram_tensor("vaugo", [128, 16 * (HD + 1)], BF16,
                                kind="ExternalOutput")
        asb_o = nc.dram_tensor("asbo", [128, 16 * HD], F16,
                               kind="ExternalOutput")

    with tile.TileContext(nc) as tc:
        with tc.tile_pool(name="wp", bufs=1) as wp, \
             tc.tile_pool(name="expp", bufs=8) as expp, \
             tc.tile_pool(name="tmpq", bufs=2) as tmppool, \
             tc.tile_pool(name="ps", bufs=1, space="PSUM") as ps:

            # ---------------- input DMAs (priority order) ----------------
            cst = wp.tile([128, 40], F32, tag="cst")
            rr = lambda d: d[:, :].rearrange("(k p) t -> p k t", p=128)
            xTc = [wp.tile([128, 4, 256], F8, tag=f"xTc{h}", name=f"xTc{h}")
                   for h in range(2)]
            wq = [wp.tile([128, 4, DIM], F8, tag=f"wq{h}", name=f"wq{h}")
                  for h in range(2)]
            xT = [wp.tile([128, 4, L], F8, tag=f"xT{h}", name=f"xT{h}")
                  for h in range(2)]
            wkv = [wp.tile([128, 4, CH], F8, tag=f"wkv{h}", name=f"wkv{h}")
                   for h in range(2)]
            dgv = wp.tile([128, 56], F32, tag="dgv")
            bpl = wp.tile([128, 8, NS], F16, tag="bpl")
            wo = wp.tile([128, 4, DIM], F16, tag="wo")
            xT_r = [rr(xTh_d), rr(xTl_d)]
            wkv_r = [rr(wkvh_d), rr(wkvl_d)]
            xTc_r = [rr(xTch_d), rr(xTcl_d)]
            wq_r = [rr(wqh_d), rr(wql_d)]
            bpl_r = bpl_d[:, :].rearrange("(t p) s -> p t s", p=128)

            # critical-path order: K0/V0 proj operands, q operands, then
            # second-half tokens, conv bias plane, remaining kv weights,
            # y-proj weights last.
            for h in range(2):
                nc.sync.dma_start(out=wkv[h][:, :, 0:256],
                                  in_=wkv_r[h][:, :, 0:256])
            nc.sync.dma_start(out=dgv, in_=dgv_d[:, :])
            nc.sync.dma_start(out=cst, in_=cst_d[:, :])
            for h in range(2):
                nc.sync.dma_start(out=xT[h][:, :, 0:512],
                                  in_=xT_r[h][:, :, 0:512])
            nc.sync.dma_start(out=bpl[:, 0:2, :], in_=bpl_r[:, 0:2, :])
            nc.sync.dma_start(out=xTc[0], in_=xTc_r[0])
            nc.sync.dma_start(out=wq[0], in_=wq_r[0])
            for h in range(2):
                nc.sync.dma_start(out=xT[h][:, :, 512:L],
                                  in_=xT_r[h][:, :, 512:L])
            nc.sync.dma_start(out=xTc[1], in_=xTc_r[1])
            nc.sync.dma_start(out=wq[1], in_=wq_r[1])
            nc.sync.dma_start(out=bpl[:, 2:8, :], in_=bpl_r[:, 2:8, :])
            for h in range(2):
                nc.sync.dma_start(out=wkv[h][:, :, 256:CH],
                                  in_=wkv_r[h][:, :, 256:CH])
            nc.sync.dma_start(out=wo,
                              in_=wo_d[:, :].rearrange("(k p) c -> p k c", p=128))

            # ---------------- persistent SBUF state ----------------
            ident = wp.tile([128, 128], F16, tag="ident")
            make_identity(nc, ident)
            # preload exp ACT table during the DMA window
            warm = wp.tile([1, 1], F32, tag="warm")
            nc.vector.memset(warm, 0.0)
            nc.scalar.activation(warm[:, :], warm[:, :], AF.Exp)

            kT2 = wp.tile([64, M], F16, tag="kT2")
            qT2 = wp.tile([64, M], F16, tag="qT2")
            vaug = wp.tile([128, 16, HD + 1], BF16, tag="vaug")
            nc.vector.memset(vaug[:, :, HD:HD + 1], 1.0)
            attnT = [wp.tile([128, 256], F16, tag=f"attnT{k}",
                              name=f"attnT{k}") for k in range(4)]
            attn_sb = [wp.tile([128, 4, HD], F16, tag=f"attn_sb{k}",
                                name=f"attn_sb{k}") for k in range(4)]
            zr = [wp.tile([128, 4], F32, tag=f"zr{k}", name=f"zr{k}")
                  for k in range(4)]
            vacc = wp.tile([128, NS], F16, tag="vacc")
            ysb = [wp.tile([128, 2, 256], F32, tag=f"ysb{h}",
                          name=f"ysb{h}") for h in range(2)]

            # zero strip for PE warm-up and PSUM zero-region matmuls
            zstrip = wp.tile([128, 512], F16, tag="zstrip")
            nc.gpsimd.memset(zstrip[:, :], 0.0)

            # conv-input buffers: only SAME-pad border cells are zeroed
            # (col 32 of rows 0-32, then row 32 cols 0-31); the interior
            # is fully overwritten by pad_copy each use.
            kvsb = []
            for i in range(2):
                kb = wp.tile([128, NPAD], F16, tag=f"kvsb{i}",
                             name=f"kvsb{i}")
                colpad = bass.AP(tensor=kb.tensor, offset=kb.offset + 32,
                                 ap=[kb.ap[0], [PADW, PADW]])
                rowpad = bass.AP(tensor=kb.tensor,
                                 offset=kb.offset + PADW * 32,
                                 ap=[kb.ap[0], [1, 32]])
                nc.gpsimd.memset(colpad, 0.0)
                nc.gpsimd.memset(rowpad, 0.0)
                kvsb.append(kb)

            # conv diag weights built on-chip: dgt[:, dg*7+tap, :] =
            # diag(dgv[:, dg*7+tap]) via stride-0-broadcast tensor_tensor
            dgt = wp.tile([128, 56, 128], F16, tag="dgt")

            def diag_op(dg, eng=None):
                if eng is nc.scalar:
                    for tap in range(7):
                        i = dg * 7 + tap
                        _tag(nc.scalar.mul(dgt[:, i, :], ident[:, :],
                                           dgv[:, i:i + 1]), f"diag({dg})")
                    return
                id_b = bass.AP(tensor=ident.tensor, offset=ident.offset,
                               ap=[ident.ap[0], [0, 7], [1, 128]])
                dg_b = bass.AP(tensor=dgv.tensor, offset=dgv.offset + 7 * dg,
                               ap=[dgv.ap[0], [1, 7], [0, 128]])
                _tag((eng or nc.gpsimd).tensor_tensor(
                    dgt[:, dg * 7:(dg + 1) * 7, :],
                    id_b, dg_b, op=ALU.mult), f"diag({dg})")

            diag_op(0, eng=nc.vector)
            diag_op(1, eng=nc.vector)
            diag_op(4)
            diag_op(5)
            diag_op(6)
            diag_op(7)
            diag_late = {}

            # PE warm-up: keep the array busy through the DMA window so the
            # p-state ramp completes before the real matmuls arrive.
            for _w in range(4):
                wmm = ps.tile([128, 512], F32, tag="kvp", bufs=1)
                nc.tensor.matmul(wmm[:, :], ident[:, :], zstrip[:, :],
                                 start=True, stop=True)

            # ---------------- helpers ----------------
            def pad_copy(dg, nh, kvp, eng):
                kb = kvsb[dg % 2]
                dst = bass.AP(tensor=kb.tensor,
                              offset=kb.offset + PADW * 16 * nh,
                              ap=[kb.ap[0], [PADW, 16], [1, 32]])
                src = kvp[:, :].rearrange("p (a b) -> p a b", b=32)
                if eng is nc.scalar:
                    _tag(eng.mul(dst, src, 1.0 / 16.0), f"pad({dg},{nh})")
                else:
                    _tag(eng.tensor_scalar_mul(dst, src, 1.0 / 16.0),
                         f"pad({dg},{nh})")

            tmpp = tmppool

            def conv(dg):
                """Conv taps 0-6 on PE (PSUM cvp); taps 7,8 + bias into an
                SBUF side-accumulator on gpsimd, merged at the K/V stt."""
                kb = kvsb[dg % 2]
                tmp = tmpp.tile([128, NS], F32, tag="tmp")
                for tap in (7, 8):
                    dy, dx = tap // 3, tap % 3
                    win = bass.AP(tensor=kb.tensor,
                                  offset=kb.offset + PADW * dy + dx,
                                  ap=[kb.ap[0], [2 * PADW, KH], [2, KW]])
                    col = 8 + (tap - 5) * 8 + dg
                    in1 = bpl[:, dg, :] if tap == 7 else tmp[:, :]
                    _tag(nc.vector.scalar_tensor_tensor(
                        tmp[:, :], win, cst[:, col:col + 1], in1,
                        op0=ALU.mult, op1=ALU.add), f"ctap({dg},{tap})")
                cvp = ps.tile([128, NS], F32, tag="kvp", bufs=1)
                for tap in range(7):
                    dy, dx = tap // 3, tap % 3
                    win = bass.AP(tensor=kb.tensor,
                                  offset=kb.offset + PADW * dy + dx,
                                  ap=[kb.ap[0], [2 * PADW, KH], [2, KW]])
                    _tag(nc.tensor.matmul(cvp[:, :], dgt[:, dg * 7 + tap, :], win,
                                     start=(tap == 0), stop=(tap == 6)),
                         f"cmm({dg},{tap})")
                return cvp, tmp

            def k_tile(t, cvp, tmp):
                # gpsimd cannot read PSUM: merge to SBUF, then cross-copy
                kacc = tmpp.tile([128, NS], F16, tag="kacc")
                _tag(nc.vector.scalar_tensor_tensor(
                    kacc[:, :], cvp[:, :], 1.0, tmp[:, :],
                    op0=ALU.mult, op1=ALU.add), f"kmerge({t})")
                _tag(nc.gpsimd.tensor_copy(kT2[:, (2 * t) * 256:(2 * t + 1) * 256],
                                      kacc[0:64, :]), f"kcopy({t},0)")
                _tag(nc.gpsimd.tensor_copy(
                    kT2[:, (2 * t + 1) * 256:(2 * t + 2) * 256],
                    kacc[64:128, :]), f"kcopy({t},1)")

            def v_tile(t, cvp, tmp):
                _tag(nc.vector.scalar_tensor_tensor(
                    vacc[:, :], cvp[:, :], 1.0, tmp[:, :],
                    op0=ALU.mult, op1=ALU.add), f"vmerge({t})")
                for gi in range(2):
                    vt = ps.tile([128, 128], F16, tag="kvp", bufs=1)
                    for sh in range(2):
                        nc.tensor.transpose(
                            vt[:, sh * 64:(sh + 1) * 64],
                            vacc[gi * 64:(gi + 1) * 64,
                                 sh * 128:(sh + 1) * 128],
                            ident[gi * 64:(gi + 1) * 64,
                                  gi * 64:(gi + 1) * 64])
                    mt0 = 4 * t + 2 * gi
                    _tag(nc.vector.tensor_copy(
                        vaug[:, mt0:mt0 + 2, 0:HD],
                        vt[:, :].rearrange("p (a b) -> p a b", b=64)),
                         f"vcopy({t},{gi})")

            # per-uq AV accumulators, one PSUM bank each so normalize
            # reads of uq never serialize against av writes of uq+1
            avt = [ps.tile([128, 4, 128], F32, tag="av", bufs=4,
                           name=f"av{u}") for u in range(4)]
            for u in range(4):
                zmm = bass.AP(tensor=avt[u].tensor, offset=avt[u].offset,
                              ap=[avt[u].ap[0], [1, 512]])
                nc.tensor.matmul(zmm, ident[:, :], zstrip[:, :],
                                 start=True, stop=True, skip_group_check=True)

            ucount = [0]

            def sc_unit(mt, uq, force_eng=None):
                st = ps.tile([128, 512], F32, tag="st", bufs=3)
                _tag(nc.tensor.matmul(st[:, :],
                                 kT2[:, mt * 128:(mt + 1) * 128],
                                 qT2[:, uq * 512:(uq + 1) * 512],
                                 start=True, stop=True), f"sc({mt},{uq})")
                ucount[0] += 1
                if force_eng == "split":
                    ex = expp.tile([128, 512], BF16, tag="ex")
                    _tag(nc.scalar.activation(ex[:, 0:256], st[:, 0:256],
                                              AF.Exp, scale=float(SCALE)),
                         f"expA({mt},{uq})")
                    exd = ex[:, 256:512].bitcast(mybir.dt.int16)
                    _tag(nc.vector.tensor_scalar(
                        exd, st[:, 256:512], 23.08294, 16248.78,
                        op0=ALU.mult, op1=ALU.add), f"expD({mt},{uq})")
                    return ex
                dve = ucount[0] % 3 == 0
                if force_eng is not None:
                    dve = force_eng == "dve"
                if dve:
                    # Schraudolph exp on DVE: bf16 bits = s*scale*184.66 +
                    # (16256 - 7.22); int16 convert, bitcast view as bf16.
                    exd = expp.tile([128, 512], mybir.dt.int16, tag="ex",
                                    name="exd")
                    _tag(nc.vector.tensor_scalar(
                        exd[:, :], st[:, :], 23.08294, 16248.78,
                        op0=ALU.mult, op1=ALU.add), f"expD({mt},{uq})")
                    return exd.bitcast(BF16)
                ex = expp.tile([128, 512], BF16, tag="ex")
                _tag(nc.scalar.activation(ex[:, :], st[:, :], AF.Exp,
                                     scale=float(SCALE)), f"expA({mt},{uq})")
                return ex

            def av_unit(mt, uq, ex):
                for j in range(4):
                    _tag(nc.tensor.matmul(avt[uq][:, j, 0:HD + 1],
                                     ex[:, j * 128:(j + 1) * 128],
                                     vaug[:, mt, :],
                                     start=False, stop=(mt == 15),
                                     skip_group_check=True), f"av({mt},{uq},{j})")

            # ---------------- q projection ----------------
            def emit_qproj(t):
                eng = nc.scalar if t % 2 == 0 else nc.vector
                qp = ps.tile([128, 512], F32, tag="st", bufs=3)
                qpv = qp[:, 0:256]
                first = True
                for s in range(2):
                    for wh, xh in ((0, 0), (0, 1), (1, 0)):
                        last = s == 1 and (wh, xh) == (1, 0)
                        _tag(nc.tensor.matmul(
                            qpv,
                            wq[wh][:, 2 * s:2 * s + 2,
                                   t * 128:(t + 1) * 128],
                            xTc[xh][:, 2 * s:2 * s + 2, :],
                            start=first, stop=last,
                            perf_mode=mybir.MatmulPerfMode.DoubleRow),
                             f"qp({t},{s})")
                        first = False
                if eng is nc.scalar:
                    nc.scalar.add(qpv, qpv, cst[:, t:t + 1])
                    nc.scalar.mul(qT2[:, (2 * t) * 256:(2 * t + 1) * 256],
                                  qp[0:64, 0:256], 1.0 / 16.0)
                else:
                    nc.vector.tensor_scalar_add(qpv, qpv, cst[:, t:t + 1])
                    nc.vector.tensor_scalar_mul(
                        qT2[:, (2 * t) * 256:(2 * t + 1) * 256],
                        qp[0:64, 0:256], 1.0 / 16.0)
                qstage = tmpp.tile([128, 256], F16, tag="kacc", name="qstage")
                if eng is nc.scalar:
                    nc.scalar.mul(qstage[64:128, :],
                                  qp[64:128, 0:256], 1.0 / 16.0)
                else:
                    nc.vector.tensor_scalar_mul(qstage[64:128, :],
                                                qp[64:128, 0:256], 1.0 / 16.0)
                nc.gpsimd.tensor_copy(qT2[:, (2 * t + 1) * 256:(2 * t + 2) * 256],
                                      qstage[64:128, :])

            # ---------------- fused P1/P2 pipeline ----------------
            kvp_store = {}

            def pad_eng(dg):
                # gpsimd cannot access PSUM; split PSUM reads DVE/ACT
                if dg % 2 == 0:
                    return nc.scalar
                return nc.vector

            def proj_op(dg, nh):
                if dg % 2 == 0:  # K tile -> st slots
                    kvp = ps.tile([128, 512], F32, tag="st", bufs=3,
                                  name="kvpk")
                else:
                    kvp = ps.tile([128, 512], F32, tag="kvp", bufs=1)
                first = True
                for s in range(2):
                    for wh, xh in ((0, 0), (0, 1), (1, 0)):
                        last = s == 1 and (wh, xh) == (1, 0)
                        _tag(nc.tensor.matmul(
                            kvp[:, :],
                            wkv[wh][:, 2 * s:2 * s + 2,
                                    dg * 128:(dg + 1) * 128],
                            xT[xh][:, 2 * s:2 * s + 2,
                                   nh * 512:(nh + 1) * 512],
                            start=first, stop=last,
                            perf_mode=mybir.MatmulPerfMode.DoubleRow),
                             f"proj({dg},{nh},{s})")
                        first = False
                kvp_store[(dg, nh)] = kvp

            def pad_op(dg, nh):
                pad_copy(dg, nh, kvp_store.pop((dg, nh)), pad_eng(dg))

            def p1_ops(t):
                dgk, dgv_ = 2 * t, 2 * t + 1
                mk = lambda f, *a: (lambda: f(*a))
                cstore = {}

                def conv_op(dg):
                    cstore[dg] = conv(dg)

                def ktail(tt):
                    k_tile(tt, *cstore.pop(2 * tt))

                def vtail(tt):
                    v_tile(tt, *cstore.pop(2 * tt + 1))

                ops = [
                    mk(proj_op, dgk, 0), mk(pad_op, dgk, 0),
                    mk(proj_op, dgv_, 0), mk(pad_op, dgv_, 0),
                    mk(proj_op, dgk, 1), mk(pad_op, dgk, 1),
                    mk(conv_op, dgk), mk(ktail, t),
                    mk(proj_op, dgv_, 1), mk(pad_op, dgv_, 1),
                    mk(conv_op, dgv_), mk(vtail, t),
                ]
                for dgx in diag_late.pop(t, ()):
                    ops.append(mk(diag_op, dgx))
                return ops

            ops0 = p1_ops(0)
            for op in ops0[0:8]:     # K0/V0 both halves
                op()
            ops0[8]()                # conv K0
            ops0[9]()                # ktail(0)
            emit_qproj(0)
            emit_qproj(1)
            ops0[10]()               # conv V0
            emit_qproj(2)
            emit_qproj(3)
            ops0[11]()               # vtail(0)
            diag_op(2, eng=nc.vector)
            diag_op(3, eng=nc.vector)

            # ---------------- normalize / attnT helpers ----------------
            def slot_of(chunk):
                h, lh = chunk // 2, chunk % 2
                return 4 * (h // 2) + 2 * lh + (h % 2)

            def norm_chunk(chunk):
                slot = slot_of(chunk)
                kk, sl = slot // 4, slot % 4
                uq, j = chunk // 4, chunk % 4
                src_v = avt[uq][:, j, 0:HD]
                src_z = avt[uq][:, j, HD:HD + 1]
                if chunk % 2 == 0:
                    _tag(nc.vector.reciprocal(zr[uq][:, j:j + 1], src_z),
                         f"recip({chunk})")
                    _tag(nc.vector.tensor_scalar_mul(
                        attn_sb[kk][:, sl, :], src_v,
                        zr[uq][:, j:j + 1]), f"nmul({chunk})")
                else:
                    _tag(nc.vector.reciprocal(zr[uq][:, j:j + 1], src_z),
                         f"recip({chunk})")
                    _tag(nc.scalar.mul(attn_sb[kk][:, sl, :], src_v,
                                  zr[uq][:, j:j + 1]), f"nmul({chunk})")

            vt2a_pre = {}

            def attnT_block(kk):
                # transposes + SBUF staging only (y matmuls emitted later)
                for lh in range(2):
                    if lh == 0:
                        vt2 = ps.tile([128, 128], F16, tag="kvp", bufs=1)
                    elif kk in vt2a_pre:
                        vt2 = vt2a_pre.pop(kk)
                    else:
                        vt2 = ps.tile([128, 128], F16, tag="av", bufs=4,
                                      name="vt2a")
                    asb = attn_sb[kk]
                    src_ap = bass.AP(
                        tensor=asb.tensor,
                        offset=asb.offset + 2 * lh * HD,
                        ap=[asb.ap[0], [1, 128]])
                    _tag(nc.tensor.transpose(vt2[:, :], src_ap, ident[:, :]),
                         f"atT({kk},{lh})")
                    if lh == 0:
                        _tag(nc.scalar.copy(
                            attnT[kk][:, 0:128], vt2[:, :]), f"atC({kk},{lh})")
                    else:
                        _tag(nc.vector.tensor_copy(
                            attnT[kk][:, 128:256], vt2[:, :]), f"atC({kk},{lh})")

            # Unit stream with av lagging sc by one; p1 micro-ops of tile
            # t+1 round-robined between tile t's units.  Groups t=0..2 run
            # mt-major; the last group runs uq-major so each uq's
            # normalize/attnT pipeline overlaps the next uq's score units.
            pending = []

            def drain_pending(keep):
                while len(pending) > keep:
                    av_unit(*pending.pop(0))

            for t in range(3):
                us = [(mt, uq) for mt in range(4 * t, 4 * t + 4)
                      for uq in range(4)]
                chunks = p1_ops(t + 1)
                nu, nch = len(us), len(chunks)
                ci = 0
                for i, u in enumerate(us):
                    mt, uq = u
                    ex = sc_unit(mt, uq)
                    pending.append((mt, uq, ex))
                    drain_pending(2)
                    want = ((i + 1) * nch) // nu
                    while ci < want:
                        chunks[ci]()
                        ci += 1

            # Last group: uq-major score stream with exps strictly
            # alternating engines; uq's normalize + attnT emitted AFTER
            # uq+1's score units so the exp cadence stays hot.
            for uq in range(4):
                for i, mt in enumerate(range(12, 16)):
                    force = "dve" if (uq * 4 + i) % 2 else "act"
                    ex = sc_unit(mt, uq, force_eng=force)
                    pending.append((mt, uq, ex))
                    drain_pending(2)
                if uq >= 1:
                    for j in range(4):
                        norm_chunk((uq - 1) * 4 + j)
                    attnT_block(uq - 1)
            drain_pending(0)
            # uq3 normalize must be emitted before its av slot is recycled
            for j in range(4):
                norm_chunk(12 + j)
            # vt2a(3) takes av slot 3 (WAR = uq3 norm reads, its true dep);
            # yp0-2 take the st slots freed by the last score units; yp3
            # takes av slot 0 (WAR = atC(0,1), long emitted).
            vt2a_pre[3] = ps.tile([128, 128], F16, tag="av", bufs=4,
                                  name="vt2a")
            yps = [ps.tile([128, 256], F32, tag="st", bufs=3,
                           name=f"yp{m}") for m in range(3)]
            yps.append(ps.tile([128, 256], F32, tag="av", bufs=4,
                               name="yp3"))

            def y_block(kk, with_copies=False):
                for m in range(4):
                    _tag(nc.tensor.matmul(
                        yps[m][:, :],
                        wo[:, kk, m * 128:(m + 1) * 128],
                        attnT[kk][:, :],
                        start=(kk == 0), stop=(kk == 3),
                        skip_group_check=True), f"y({kk},{m})")
                    if with_copies:
                        if m % 2 == 0:
                            _tag(nc.vector.tensor_scalar_add(
                                ysb[m // 2][:, m % 2, :], yps[m][:, :],
                                cst[:, 4 + m:5 + m]), f"ysb({m})")
                        else:
                            _tag(nc.scalar.add(
                                ysb[m // 2][:, m % 2, :], yps[m][:, :],
                                cst[:, 4 + m:5 + m]), f"ysb({m})")
                        if m == 1:
                            nc.sync.dma_start(
                                out=y_d[0:256, :].rearrange(
                                    "(m p) t -> p m t", p=128),
                                in_=ysb[0][:, :, :])

            for kk in range(3):
                y_block(kk)
            attnT_block(3)
            y_block(3, with_copies=True)
            if _dbg:
                nc.sync.dma_start(out=kT2_o[:, :], in_=kT2[:, :])
                nc.sync.dma_start(out=qT2_o[:, :], in_=qT2[:, :])
                nc.sync.dma_start(out=vaug_o[:, :],
                                  in_=vaug[:, :, :].rearrange("p a b -> p (a b)"))
                nc.sync.dma_start(out=asb_o[:, :],
                                  in_=attn_sb[:, :, :].rearrange("p a b -> p (a b)"))
            nc.sync.dma_start(
                out=y_d[256:512, :].rearrange("(m p) t -> p m t", p=128),
                in_=ysb[1][:, :, :])

    nc.finalize()
    return nc


def _get_program():
    if "nc" not in _NC_CACHE:
        _NC_CACHE["nc"] = _build_program()
    return _NC_CACHE["nc"]


def _hilo8(a):
    """[rows, cols] f32 -> (hi, lo) fp8 e4m3 arrays."""
    import ml_dtypes
    f8 = ml_dtypes.float8_e4m3
    h = a.astype(f8)
    l = (a - h.astype(np.float32)).astype(f8)
    return np.ascontiguousarray(h), np.ascontiguousarray(l)


def _host_prep(x, wq, bq, wkv, bkv, dw_kernel, dw_bias, wo, bo):
    """Build the 8 per-core input maps."""
    x = np.ascontiguousarray(np.asarray(x, np.float32))
    wqh, wql = _hilo8(np.asarray(wq, np.float32) * 16.0)
    wo16 = np.asarray(wo, np.float32).astype(np.float16)
    bq = np.asarray(bq, np.float32)
    bkv = np.asarray(bkv, np.float32)
    dw_bias = np.asarray(dw_bias, np.float32)
    bo = np.asarray(bo, np.float32)
    dww = np.asarray(dw_kernel, np.float32).reshape(9, CH).T.copy()  # [1024, 9]

    # channel-tile processing order K0 V0 K1 V1 ... ; dg -> channel base
    dg_base = []
    for t in range(4):
        dg_base += [t * 128, DIM + t * 128]

    # wkv columns reordered to dg order
    wkv_f = np.asarray(wkv, np.float32)
    wkv_o = np.empty((DIM, CH), np.float32)
    for dg in range(8):
        b0 = dg_base[dg]
        wkv_o[:, dg * 128:(dg + 1) * 128] = wkv_f[:, b0:b0 + 128]
    wkvh, wkvl = _hilo8(wkv_o * 16.0)

    # bias plane: dw_bias + bkv * sum(valid taps), SAME padding aware,
    # rows in dg order
    oy = np.arange(KH)
    valid_y = (2 * oy[:, None] + np.arange(3)[None, :]) < H      # [16, 3]
    valid_x = valid_y.copy()
    wsum = np.zeros((CH, KH, KW), np.float32)
    for tap in range(9):
        dy, dx = tap // 3, tap % 3
        m2 = np.outer(valid_y[:, dy], valid_x[:, dx]).astype(np.float32)
        wsum += dww[:, tap][:, None, None] * m2[None, :, :]
    bpl_full = (dw_bias[:, None] + bkv[:, None] * wsum.reshape(CH, NS))
    bpl16 = np.empty((CH, NS), np.float16)
    for dg in range(8):
        b0 = dg_base[dg]
        bpl16[dg * 128:(dg + 1) * 128] = bpl_full[b0:b0 + 128]

    # conv diag values [128, 56] f32, dg-major then tap (taps 0-6 on PE)
    dgv = np.zeros((128, 56), np.float32)
    for dg in range(8):
        b0 = dg_base[dg]
        for tap in range(7):
            dgv[:, dg * 7 + tap] = dww[b0:b0 + 128, tap]

    cst = np.zeros((128, 40), np.float32)
    cst[:, 0:4] = 16.0 * bq.reshape(4, 128).T
    cst[:, 4:8] = bo.reshape(4, 128).T
    for dg in range(8):
        b0 = dg_base[dg]
        for tap in range(5, 9):
            cst[:, 8 + (tap - 5) * 8 + dg] = dww[b0:b0 + 128, tap]

    in_maps = []
    for c in range(8):
        b, j = c // 4, c % 4
        xh, xl = _hilo8(np.ascontiguousarray(x[b].reshape(L, DIM).T))
        in_maps.append({
            "xTh": xh, "xTl": xl,
            "xTch": np.ascontiguousarray(xh[:, j * 256:(j + 1) * 256]),
            "xTcl": np.ascontiguousarray(xl[:, j * 256:(j + 1) * 256]),
            "wkvh": wkvh, "wkvl": wkvl, "wqh": wqh, "wql": wql,
            "wo": wo16,
            "bpl": bpl16, "dgv": dgv, "cst": cst,
        })
    return in_maps


def kernel(**inputs) -> np.ndarray:
    nc = _get_program()
    in_maps = _host_prep(**inputs)
    res = run_bass_kernel_spmd(nc, in_maps, core_ids=list(range(8)))
    out = np.zeros((B, H, W, DIM), np.float32)
    flat = out.reshape(B, L, DIM)
    for c in range(8):
        b, j = c // 4, c % 4
        flat[b, j * 256:(j + 1) * 256, :] = res.results[c]["y"].T
    return out
